# revision 1
# baseline (speedup 1.0000x reference)
"""GCN (3x GCNConv + global max pool + MLP) on 8 Trainium2 NeuronCores.

Strategy (data-parallel over graphs, per sharding hint):
 - Nodes laid out graph-padded: each graph gets a fixed slot of L_PAD columns;
   core c owns graphs [8c, 8c+8) -> M_PAD = 8*L_PAD padded node slots.
 - Per layer: p = h @ W computed for own nodes, AllGather -> replicated p table,
   per-edge gather of p[src] rows via dma_gather (edges sharded by dst core,
   grouped by 128-dst block), aggregation as PE matmuls with host-precomputed
   weighted one-hot selection matrices S (norm folded in, self-loops included,
   bias via a rank-1 ones-matmul), relu on evacuation, transpose to feature-major
   for the next layer's lhsT.
 - Pooling: per-graph column-slices reduce_max (pad cols are exactly 0, relu
   output >= 0, so padding never changes the max). Pooled vectors AllGathered,
   MLP head computed redundantly on every core.
"""
import os
import sys
import numpy as np

for _p in ('/opt/trn_rl_repo', '/root/.axon_site/_ro/trn_rl_repo'):
    if os.path.isdir(_p) and _p not in sys.path:
        sys.path.insert(0, _p)

N_CORES = 8
N_NODES = 50000
D = 320
N_GRAPHS = 64
GPC = N_GRAPHS // N_CORES  # graphs per core


def _preprocess(x, edge_index, batch):
    """Build per-core gather indices, selection matrices and layouts."""
    batch = np.asarray(batch).astype(np.int64)
    src = np.asarray(edge_index[0]).astype(np.int64)
    dst = np.asarray(edge_index[1]).astype(np.int64)
    counts = np.bincount(batch, minlength=N_GRAPHS)
    L_PAD = max(896, int(-(-counts.max() // 128)) * 128)
    M_PAD = GPC * L_PAD
    TOT = N_CORES * M_PAD
    HALF = TOT // 2
    assert HALF <= 32767, (L_PAD, HALF)
    NB = M_PAD // 128  # dst blocks per core

    gstart = np.zeros(N_GRAPHS, np.int64)
    gstart[1:] = np.cumsum(counts)[:-1]
    n_ar = np.arange(N_NODES, dtype=np.int64)
    # within-graph relabeling: deal nodes (sorted by in-degree desc) round-robin
    # into the graph's L_PAD/128 blocks so per-block in-degree is balanced,
    # which minimizes the uniform per-(block,half) tile count K_FIX.
    NBLK = L_PAD // 128
    indeg = np.bincount(dst, minlength=N_NODES) + 1
    order_bal = np.lexsort((-indeg, batch))
    r = n_ar - gstart[batch[order_bal]]
    posg = (r % NBLK) * 128 + r // NBLK
    pos_in_graph = np.empty(N_NODES, np.int64)
    pos_in_graph[order_bal] = posg
    pos = (batch // GPC) * M_PAD + (batch % GPC) * L_PAD + pos_in_graph

    deg = np.bincount(dst, minlength=N_NODES).astype(np.float64) + 1.0
    dinv = 1.0 / np.sqrt(deg)

    # self-loops are applied locally (h += dinv^2 * p_own), not gathered
    ms = src
    mt = dst
    w = (dinv[src] * dinv[dst]).astype(np.float32)

    ms_pos = pos[ms]
    mt_pos = pos[mt]
    core = mt_pos // M_PAD
    lb = (mt_pos % M_PAD) // 128
    dl = mt_pos % 128
    half = ms_pos // HALF
    idxl = (ms_pos % HALF).astype(np.int64)

    key = (core * NB + lb) * 2 + half
    order = np.argsort(key, kind='stable')
    key_s = key[order]
    idxl_s = idxl[order]
    dl_s = dl[order]
    w_s = w[order]

    nkeys = N_CORES * NB * 2
    kcounts = np.bincount(key_s, minlength=nkeys).reshape(N_CORES, NB, 2)
    k_req = -(-kcounts // 128)  # ceil
    K_FIX = k_req.max(axis=0)  # [NB, 2] uniform across cores
    toff = np.zeros((NB, 2), np.int64)
    flat_k = K_FIX.reshape(-1)
    toff.reshape(-1)[1:] = np.cumsum(flat_k)[:-1]
    T_TOTAL = int(flat_k.sum())

    # rank of each message within its (core, lb, half) group
    kstart = np.zeros(nkeys, np.int64)
    kstart[1:] = np.cumsum(np.bincount(key_s, minlength=nkeys))[:-1]
    rank = np.arange(len(key_s)) - kstart[key_s]

    core_s = key_s // (NB * 2)
    lbh = key_s % (NB * 2)
    lb_s = lbh // 2
    half_s = lbh % 2
    t_glob = toff[lb_s, half_s] + rank // 128  # global tile id
    p_slot = rank % 128

    S_all = np.zeros((N_CORES, 128, T_TOTAL * 128), np.float32)
    S_all[core_s, p_slot, t_glob * 128 + dl_s] = w_s

    # dma_gather flat order: flat[i] = idxs[i % 16, i // 16]; i is the
    # message index within the call starting at tile toff[lb, half].
    idx_all = np.zeros((N_CORES, 16, T_TOTAL * 8), np.int16)
    i_in_call = (rank // 128 - (t_glob - toff[lb_s, half_s])) * 0 + rank  # = rank
    colbase = toff[lb_s, half_s] * 8
    idx_all[core_s, i_in_call % 16, colbase + i_in_call // 16] = idxl_s.astype(np.int16)
    idx_rep = np.tile(idx_all, (1, 8, 1))  # [N_CORES, 128, T*8]

    # real-node mask per core [1, M_PAD]
    real = np.zeros(TOT, np.float32)
    real[pos] = 1.0
    mask = real.reshape(N_CORES, 1, M_PAD)

    # dinv^2 per padded slot, laid out [core, 128, NB] (partition p, block b)
    d2_flat = np.zeros(TOT, np.float32)
    d2_flat[pos] = (dinv * dinv).astype(np.float32)
    dinv2 = d2_flat.reshape(N_CORES, NB, 128).transpose(0, 2, 1).copy()

    # xT padded per core [D, M_PAD]
    x = np.asarray(x, dtype=np.float32)
    xT_pad = np.zeros((D, TOT), np.float32)
    xT_pad[:, pos] = x.T
    xT_own = np.stack([xT_pad[:, c * M_PAD:(c + 1) * M_PAD] for c in range(N_CORES)])

    meta = dict(L_PAD=L_PAD, M_PAD=M_PAD, TOT=TOT, HALF=HALF, NB=NB,
                K_FIX=K_FIX, toff=toff, T_TOTAL=T_TOTAL)
    return meta, S_all, idx_rep, mask, xT_own, dinv2


def _build_bass(meta, weights, repeat=1):
    from concourse import mybir, bacc
    import concourse.tile as tile
    from concourse.masks import make_identity

    L_PAD = meta['L_PAD']; M_PAD = meta['M_PAD']; TOT = meta['TOT']
    HALF = meta['HALF']; NB = meta['NB']
    K_FIX = meta['K_FIX']; toff = meta['toff']; T_TOTAL = meta['T_TOTAL']
    f32 = mybir.dt.float32
    f32r = mybir.dt.float32r
    i16 = mybir.dt.int16
    FCH = [(0, 128), (128, 128), (256, 64)]  # feature chunks of 320

    nc = bacc.Bacc("TRN2", target_bir_lowering=False, debug=False,
                   num_devices=N_CORES, num_swdge_queues=4)

    # ---- IO ----
    S_t = nc.dram_tensor("S_all", [128, T_TOTAL * 128], f32r, kind="ExternalInput")
    idx_t = nc.dram_tensor("idx_all", [128, T_TOTAL * 8], i16, kind="ExternalInput")
    mask_t = nc.dram_tensor("mask", [1, M_PAD], f32r, kind="ExternalInput")
    dinv2_t = nc.dram_tensor("dinv2", [128, NB], f32, kind="ExternalInput")
    xT_t = nc.dram_tensor("xT", [D, M_PAD], f32r, kind="ExternalInput")
    W_t = [nc.dram_tensor(f"W{k}", [D, D], f32r, kind="ExternalInput") for k in (1, 2, 3)]
    brow_t = [nc.dram_tensor(f"brow{k}", [1, D], f32r, kind="ExternalInput") for k in (1, 2, 3)]
    Wf1_t = nc.dram_tensor("Wf1", [320, 256], f32, kind="ExternalInput")
    bf1_t = nc.dram_tensor("bf1c", [128, 2], f32, kind="ExternalInput")
    Wf2_t = nc.dram_tensor("Wf2", [256, 16], f32, kind="ExternalInput")
    bf2_t = nc.dram_tensor("bf2c", [16, 1], f32, kind="ExternalInput")
    Wf3_t = nc.dram_tensor("Wf3", [16, 1], f32, kind="ExternalInput")
    out_t = nc.dram_tensor("out", [1, N_GRAPHS], f32, kind="ExternalOutput")
    bf3_val = float(np.asarray(weights['bf3']).reshape(-1)[0])

    p_own = [nc.dram_tensor(f"p_own{k}", [M_PAD, D], f32r, kind="Internal")
             for k in range(3)]
    p_full = [nc.dram_tensor(f"p_full{k}", [TOT, D], f32r, kind="Internal",
                             addr_space="Shared") for k in range(3)]
    pooled_own = nc.dram_tensor("pooled_own", [D, GPC], f32, kind="Internal")
    pooled_all = nc.dram_tensor("pooled_all", [N_CORES, D, GPC], f32,
                                kind="Internal", addr_space="Shared")

    RG = [list(range(N_CORES))]
    KMAXB = int((K_FIX[:, 0] + K_FIX[:, 1]).max())

    with tile.TileContext(nc) as tc:
        with tc.tile_pool(name="persist", bufs=1) as pp, \
             tc.tile_pool(name="gpool", bufs=2) as gp, \
             tc.tile_pool(name="spool", bufs=2) as sp, \
             tc.tile_pool(name="hpool", bufs=2) as hp, \
             tc.tile_pool(name="psum_a", bufs=2, space="PSUM") as pa, \
             tc.tile_pool(name="psum_t", bufs=2, space="PSUM") as pt, \
             tc.tile_pool(name="psum_f", bufs=2, space="PSUM") as pf:

            ident_f = pp.tile([128, 128], f32, tag="ident_f")
            make_identity(nc, ident_f[:])
            ident = pp.tile([128, 128], f32r, tag="ident")
            nc.vector.tensor_copy(ident[:], ident_f[:])
            idx_sb = pp.tile([128, T_TOTAL * 8], i16, tag="idx")
            nc.sync.dma_start(idx_sb[:], idx_t[:, :])
            d2_sb = pp.tile([128, NB], f32, tag="d2")
            nc.sync.dma_start(d2_sb[:], dinv2_t[:, :])
            brow_sb = pp.tile([1, 3 * D], f32r, tag="brow")
            for k in range(3):
                nc.sync.dma_start(brow_sb[:, k * D:(k + 1) * D], brow_t[k][:, :])

            # persistent transposed features hT (3 chunks)
            hT = [pp.tile([cl, M_PAD], f32r, tag=f"hT{ci}", name=f"hT{ci}")
                  for ci, (cs, cl) in enumerate(FCH)]
            for ci, (cs, cl) in enumerate(FCH):
                nc.sync.dma_start(hT[ci][:], xT_t[cs:cs + cl, :])

            # W chunks for the current layer (rhs [128,320] x3), reloaded per layer
            def feature_matmul_phase(layer):
                """p_own[layer] = h @ W[layer] for own nodes (h given by hT)."""
                wl = []
                for ci, (cs, cl) in enumerate(FCH):
                    wt = hp.tile([cl, D], f32r, tag=f"wch{ci}", name=f"wch{ci}", bufs=1)
                    nc.sync.dma_start(wt[:], W_t[layer][cs:cs + cl, :])
                    wl.append(wt)
                for nb in range(NB):
                    ps = pf.tile([128, D], f32, tag="pfeat")
                    for ci, (cs, cl) in enumerate(FCH):
                        nc.tensor.matmul(
                            ps[:],
                            lhsT=hT[ci][:, nb * 128:(nb + 1) * 128],
                            rhs=wl[ci][:],
                            start=(ci == 0), stop=(ci == 2))
                    pev = hp.tile([128, D], f32r, tag="pev")
                    nc.any.tensor_copy(pev[:], ps[:])
                    nc.sync.dma_start(p_own[layer][nb * 128:(nb + 1) * 128, :], pev[:])
                nc.gpsimd.collective_compute(
                    "AllGather", mybir.AluOpType.bypass, replica_groups=RG,
                    ins=[p_own[layer][:, :]], outs=[p_full[layer][:, :]])

            qn = [0]

            def agg_phase(layer):
                """hT = relu(Ahat @ p_full[layer] + b) transposed, per dst block."""
                table = p_full[layer]
                for b in range(NB):
                    k0, k1 = int(K_FIX[b, 0]), int(K_FIX[b, 1])
                    ktot = k0 + k1
                    t0 = int(toff[b, 0])
                    g = gp.tile([128, KMAXB, D], f32r, tag="g")
                    for hh, (kh, th) in enumerate(((k0, int(toff[b, 0])),
                                                   (k1, int(toff[b, 1])))):
                        if kh == 0:
                            continue
                        koff = 0 if hh == 0 else k0
                        nc.gpsimd.dma_gather(
                            out_ap=g[:, koff:koff + kh, :],
                            in_ap=table[hh * HALF:(hh + 1) * HALF, :],
                            idxs_ap=idx_sb[:, th * 8:(th + kh) * 8],
                            num_idxs=kh * 128,
                            num_idxs_reg=kh * 128,
                            elem_size=D,
                            single_packet=False,
                            queue_num=qn[0] % 4)
                        qn[0] += 1
                    s_sb = sp.tile([128, KMAXB * 128], f32r, tag="s")
                    nc.sync.dma_start(s_sb[:, :ktot * 128],
                                      S_t[:, t0 * 128:(t0 + ktot) * 128])
                    mb_sb = hp.tile([1, 128], f32r, tag="maskb")
                    nc.sync.dma_start(mb_sb[:], mask_t[:, b * 128:(b + 1) * 128])
                    ps = pa.tile([128, D], f32, tag="pagg")
                    nc.tensor.matmul(
                        ps[:],
                        lhsT=mb_sb[:],
                        rhs=brow_sb[:, layer * D:(layer + 1) * D],
                        start=True, stop=False)
                    for t in range(ktot):
                        nc.tensor.matmul(
                            ps[:],
                            lhsT=s_sb[:, t * 128:(t + 1) * 128],
                            rhs=g[:, t, :],
                            start=False, stop=(t == ktot - 1))
                    pblk = hp.tile([128, D], f32r, tag="pblk",
                                   name=f"pblk_{layer}_{b}")
                    nc.sync.dma_start(
                        pblk[:], p_own[layer][b * 128:(b + 1) * 128, :])
                    slt = hp.tile([128, D], f32r, tag="slt",
                                  name=f"slt_{layer}_{b}")
                    nc.vector.tensor_scalar_mul(slt[:], pblk[:],
                                                d2_sb[:, b:b + 1])
                    htmp = hp.tile([128, D], f32r, tag="htmp")
                    nc.vector.tensor_tensor(out=htmp[:], in0=ps[:], in1=slt[:],
                                            op=mybir.AluOpType.add)
                    nc.vector.tensor_scalar_max(htmp[:], htmp[:], 0.0)
                    for ci, (cs, cl) in enumerate(FCH):
                        tp = pt.tile([128, 128], f32r, tag="tr")
                        nc.tensor.transpose(tp[:cl, :], htmp[:, cs:cs + cl],
                                            identity=ident[:])
                        nc.any.tensor_copy(hT[ci][:, b * 128:(b + 1) * 128],
                                           tp[:cl, :])

            # ---- network ----
            import contextlib
            loop_ctx = tc.For_i(0, repeat, 1) if repeat > 1 else contextlib.nullcontext()
            with loop_ctx:
                feature_matmul_phase(0)  # p1 = x @ W1
                agg_phase(0)             # h1
                feature_matmul_phase(1)  # p2 = h1 @ W2
                agg_phase(1)             # h2
                feature_matmul_phase(2)  # p3 = h2 @ W3
                agg_phase(2)             # h3 (lives in hT)

            # ---- global max pool ----
            for ci, (cs, cl) in enumerate(FCH):
                gt = hp.tile([cl, GPC], f32, tag=f"gt{ci}", name=f"gt{ci}", bufs=1)
                for j in range(GPC):
                    nc.vector.reduce_max(
                        gt[:, j:j + 1], hT[ci][:, j * L_PAD:(j + 1) * L_PAD],
                        axis=mybir.AxisListType.X)
                nc.sync.dma_start(pooled_own[cs:cs + cl, :], gt[:])
            nc.gpsimd.collective_compute(
                "AllGather", mybir.AluOpType.bypass, replica_groups=RG,
                ins=[pooled_own[:, :]], outs=[pooled_all[:, :, :]])

            # gT_full chunks [cl, 64]
            gT = []
            for ci, (cs, cl) in enumerate(FCH):
                gtile = hp.tile([cl, N_GRAPHS], f32, tag=f"gTf{ci}", name=f"gTf{ci}", bufs=1)
                for cc in range(N_CORES):
                    nc.sync.dma_start(gtile[:, cc * GPC:(cc + 1) * GPC],
                                      pooled_all[cc, cs:cs + cl, :])
                gT.append(gtile)

            # ---- MLP head (transposed): z1T[256,64] ----
            wf1 = []
            for mi in range(2):
                for ci, (cs, cl) in enumerate(FCH):
                    t = hp.tile([cl, 128], f32, tag=f"wf1_{mi}_{ci}", name=f"wf1_{mi}_{ci}", bufs=1)
                    nc.sync.dma_start(t[:], Wf1_t[cs:cs + cl, mi * 128:(mi + 1) * 128])
                    wf1.append(t)
            bf1sb = hp.tile([128, 2], f32, tag="bf1", bufs=1)
            nc.sync.dma_start(bf1sb[:], bf1_t[:, :])
            h1T = []
            for mi in range(2):
                ps = pf.tile([128, N_GRAPHS], f32, tag="pfeat")
                for ci in range(3):
                    nc.tensor.matmul(ps[:], lhsT=wf1[mi * 3 + ci][:],
                                     rhs=gT[ci][:],
                                     start=(ci == 0), stop=(ci == 2))
                h = hp.tile([128, N_GRAPHS], f32, tag=f"h1T{mi}", name=f"h1T{mi}", bufs=1)
                nc.vector.tensor_scalar(h[:], ps[:],
                                        bf1sb[:, mi:mi + 1], 0.0,
                                        op0=mybir.AluOpType.add,
                                        op1=mybir.AluOpType.max)
                h1T.append(h)
            # z2T [16, 64]
            wf2 = []
            for mi in range(2):
                t = hp.tile([128, 16], f32, tag=f"wf2_{mi}", name=f"wf2_{mi}", bufs=1)
                nc.sync.dma_start(t[:], Wf2_t[mi * 128:(mi + 1) * 128, :])
                wf2.append(t)
            bf2sb = hp.tile([16, 1], f32, tag="bf2", bufs=1)
            nc.sync.dma_start(bf2sb[:], bf2_t[:, :])
            ps2 = pf.tile([16, N_GRAPHS], f32, tag="pfeat")
            for mi in range(2):
                nc.tensor.matmul(ps2[:], lhsT=wf2[mi][:],
                                 rhs=h1T[mi][:],
                                 start=(mi == 0), stop=(mi == 1))
            h2T = hp.tile([16, N_GRAPHS], f32, tag="h2T", bufs=1)
            nc.vector.tensor_scalar(h2T[:], ps2[:], bf2sb[:, 0:1], 0.0,
                                    op0=mybir.AluOpType.add,
                                    op1=mybir.AluOpType.max)
            # z3 [1, 64]
            wf3 = hp.tile([16, 1], f32, tag="wf3", bufs=1)
            nc.sync.dma_start(wf3[:], Wf3_t[:, :])
            ps3 = pf.tile([1, N_GRAPHS], f32, tag="pfeat")
            nc.tensor.matmul(ps3[:], lhsT=wf3[:],
                             rhs=h2T[:], start=True, stop=True)
            osb = hp.tile([1, N_GRAPHS], f32, tag="osb", bufs=1)
            nc.vector.tensor_scalar(osb[:], ps3[:], bf3_val, None,
                                    op0=mybir.AluOpType.add)
            nc.sync.dma_start(out_t[:, :], osb[:])

    nc.compile()
    return nc


def _make_runner(nc, in_maps):
    """Build a reusable jitted SPMD executor for `nc` (axon/PJRT path).

    Returns (run_fn, out_names, out_avals): run_fn() executes once and
    returns the list of per-core result dicts.
    """
    import jax
    import numpy as np
    from jax.experimental.shard_map import shard_map
    from jax.sharding import Mesh, NamedSharding, PartitionSpec
    from concourse import bass2jax, mybir

    bass2jax.install_neuronx_cc_hook()
    n_cores = len(in_maps)
    partition_name = nc.partition_id_tensor.name if nc.partition_id_tensor else None
    in_names, out_names, out_avals, zero_outs = [], [], [], []
    for alloc in nc.m.functions[0].allocations:
        if not isinstance(bass2jax.mybir.MemoryLocationSet, type) or True:
            pass
        if not isinstance(alloc, mybir.MemoryLocationSet):
            continue
        name = alloc.memorylocations[0].name
        if alloc.kind == "ExternalInput":
            if name != partition_name:
                in_names.append(name)
        elif alloc.kind == "ExternalOutput":
            shape = tuple(alloc.tensor_shape)
            dtype = mybir.dt.np(alloc.dtype)
            out_names.append(name)
            out_avals.append(jax.core.ShapedArray(shape, dtype))
            zero_outs.append(np.zeros(shape, dtype))
    n_params = len(in_names)
    n_outs = len(out_avals)
    all_in_names = list(in_names) + list(out_names)
    if partition_name is not None:
        all_in_names.append(partition_name)
    donate = tuple(range(n_params, n_params + n_outs))

    def _body(*args):
        operands = list(args)
        if partition_name is not None:
            operands.append(bass2jax.partition_id_tensor())
        outs = bass2jax._bass_exec_p.bind(
            *operands,
            out_avals=tuple(out_avals),
            in_names=tuple(all_in_names),
            out_names=tuple(out_names),
            lowering_input_output_aliases=(),
            sim_require_finite=True,
            sim_require_nnan=True,
            nc=nc,
        )
        return tuple(outs)

    devices = jax.devices()[:n_cores]
    mesh = Mesh(np.asarray(devices), ("core",))
    in_specs = (PartitionSpec("core"),) * (n_params + n_outs)
    out_specs = (PartitionSpec("core"),) * len(out_names)
    sharded = jax.jit(
        shard_map(_body, mesh=mesh, in_specs=in_specs, out_specs=out_specs,
                  check_rep=False),
        donate_argnums=donate, keep_unused=True)
    sh = NamedSharding(mesh, PartitionSpec("core"))
    concat_in = [
        jax.device_put(
            np.concatenate([np.asarray(in_maps[c][nm]) for c in range(n_cores)],
                           axis=0), sh)
        for nm in in_names
    ]

    def run_fn():
        zeros = [np.zeros((n_cores * z.shape[0], *z.shape[1:]), z.dtype)
                 for z in zero_outs]
        out_arrs = sharded(*concat_in, *zeros)
        out_arrs = [np.asarray(o) for o in out_arrs]
        return [
            {nm: out_arrs[i].reshape(n_cores, *out_avals[i].shape)[c]
             for i, nm in enumerate(out_names)}
            for c in range(n_cores)
        ]

    return run_fn, out_names, out_avals


def prepare(inputs, repeat=1):
    """Preprocess + build + compile; returns a reusable run_fn."""
    meta, S_all, idx_rep, mask, xT_own, dinv2 = _preprocess(
        inputs['x'], inputs['edge_index'], inputs['batch'])
    nc = _build_bass(meta, inputs, repeat=repeat)
    in_maps = _make_in_maps(inputs, S_all, idx_rep, mask, xT_own, dinv2)
    run_fn, _, _ = _make_runner(nc, in_maps)
    return run_fn


def _make_in_maps(inputs, S_all, idx_rep, mask, xT_own, dinv2):
    in_maps = []
    for c in range(N_CORES):
        m = {
            "S_all": S_all[c],
            "idx_all": idx_rep[c],
            "mask": mask[c],
            "dinv2": np.ascontiguousarray(dinv2[c]),
            "xT": np.ascontiguousarray(xT_own[c]),
            "Wf1": np.asarray(inputs['Wf1'], np.float32),
            "bf1c": np.ascontiguousarray(
                np.asarray(inputs['bf1'], np.float32).reshape(2, 128).T),
            "Wf2": np.asarray(inputs['Wf2'], np.float32),
            "bf2c": np.asarray(inputs['bf2'], np.float32).reshape(16, 1),
            "Wf3": np.asarray(inputs['Wf3'], np.float32),
        }
        for k in (1, 2, 3):
            m[f"W{k}"] = np.asarray(inputs[f'W{k}'], np.float32)
            m[f"brow{k}"] = np.asarray(inputs[f'b{k}'], np.float32).reshape(1, D)
        in_maps.append(m)
    return in_maps


def kernel(**inputs):
    meta, S_all, idx_rep, mask, xT_own, dinv2 = _preprocess(
        inputs['x'], inputs['edge_index'], inputs['batch'])
    nc = _build_bass(meta, inputs)
    in_maps = _make_in_maps(inputs, S_all, idx_rep, mask, xT_own, dinv2)
    from concourse.bass_utils import run_bass_kernel_spmd
    res = run_bass_kernel_spmd(nc, in_maps, core_ids=list(range(N_CORES)),
                               trace=False)
    out = np.asarray(res.results[0]["out"]).reshape(1, N_GRAPHS)
    return out.T.copy()



# revision 9
# speedup vs baseline: 1.0760x; 1.0760x over previous
"""GCN (3x GCNConv + global max pool + MLP) on 8 Trainium2 NeuronCores.

Strategy (data-parallel over graphs, per sharding hint):
 - Nodes laid out graph-padded: each graph gets a fixed slot of L_PAD columns;
   core c owns graphs [8c, 8c+8) -> M_PAD = 8*L_PAD padded node slots.
 - bf16 data path: p table, gathers, S matrices, hT all bf16 (PSUM math fp32).
   Table rows padded to 384 cols so gather elem bytes (768) is a 256 multiple.
 - Per layer: p = h @ W computed for own nodes (kept in SBUF + written to
   DRAM), AllGather -> replicated p table, per-edge gather of p[src] rows via
   dma_gather (edges sharded by dst core, grouped by 128-dst block),
   aggregation as PE matmuls with host-precomputed weighted one-hot selection
   matrices S (norm folded in, bias via a rank-1 mask-matmul), self-loop term
   from the SBUF-resident p_own, relu on evacuation, transpose to
   feature-major for the next layer's lhsT.
 - Pooling: per-graph column-slices reduce_max (pad cols are exactly 0, relu
   output >= 0, so padding never changes the max). Pooled vectors AllGathered,
   MLP head computed redundantly on every core.
"""
import os
import sys
import numpy as np

for _p in ('/opt/trn_rl_repo', '/root/.axon_site/_ro/trn_rl_repo'):
    if os.path.isdir(_p) and _p not in sys.path:
        sys.path.insert(0, _p)

N_CORES = 8
N_NODES = 50000
D = 320
DP = 384  # padded table row (bf16 -> 768B, multiple of 256)
N_GRAPHS = 64
GPC = N_GRAPHS // N_CORES  # graphs per core


def _preprocess(x, edge_index, batch):
    """Build per-core gather indices, selection matrices and layouts."""
    batch = np.asarray(batch).astype(np.int64)
    src = np.asarray(edge_index[0]).astype(np.int64)
    dst = np.asarray(edge_index[1]).astype(np.int64)
    counts = np.bincount(batch, minlength=N_GRAPHS)
    L_PAD = max(896, int(-(-counts.max() // 128)) * 128)
    M_PAD = GPC * L_PAD
    TOT = N_CORES * M_PAD
    HALF = TOT // 2
    assert HALF <= 32767, (L_PAD, HALF)
    NB = M_PAD // 128  # dst blocks per core

    gstart = np.zeros(N_GRAPHS, np.int64)
    gstart[1:] = np.cumsum(counts)[:-1]
    n_ar = np.arange(N_NODES, dtype=np.int64)
    # within-graph relabeling: deal nodes (sorted by in-degree desc) round-robin
    # into the graph's L_PAD/128 blocks so per-block in-degree is balanced,
    # which minimizes the uniform per-(block,half) tile count K_FIX.
    NBLK = L_PAD // 128
    indeg = np.bincount(dst, minlength=N_NODES) + 1
    order_bal = np.lexsort((-indeg, batch))
    r = n_ar - gstart[batch[order_bal]]
    posg = (r % NBLK) * 128 + r // NBLK
    pos_in_graph = np.empty(N_NODES, np.int64)
    pos_in_graph[order_bal] = posg
    pos = (batch // GPC) * M_PAD + (batch % GPC) * L_PAD + pos_in_graph

    deg = np.bincount(dst, minlength=N_NODES).astype(np.float64) + 1.0
    dinv = 1.0 / np.sqrt(deg)

    # self-loops are applied locally (h += dinv^2 * p_own), not gathered
    ms = src
    mt = dst
    w = (dinv[src] * dinv[dst]).astype(np.float32)

    ms_pos = pos[ms]
    mt_pos = pos[mt]
    core = mt_pos // M_PAD
    lb = (mt_pos % M_PAD) // 128
    dl = mt_pos % 128
    half = ms_pos // HALF
    idxl = (ms_pos % HALF).astype(np.int64)

    key = (core * NB + lb) * 2 + half
    order = np.argsort(key, kind='stable')
    key_s = key[order]
    idxl_s = idxl[order]
    dl_s = dl[order]
    w_s = w[order]

    nkeys = N_CORES * NB * 2
    kcounts = np.bincount(key_s, minlength=nkeys).reshape(N_CORES, NB, 2)
    k_req = -(-kcounts // 128)  # ceil
    K_FIX = k_req.max(axis=0)  # [NB, 2] uniform across cores
    toff = np.zeros((NB, 2), np.int64)
    flat_k = K_FIX.reshape(-1)
    toff.reshape(-1)[1:] = np.cumsum(flat_k)[:-1]
    T_TOTAL = int(flat_k.sum())

    # rank of each message within its (core, lb, half) group
    kstart = np.zeros(nkeys, np.int64)
    kstart[1:] = np.cumsum(np.bincount(key_s, minlength=nkeys))[:-1]
    rank = np.arange(len(key_s)) - kstart[key_s]

    core_s = key_s // (NB * 2)
    lbh = key_s % (NB * 2)
    lb_s = lbh // 2
    half_s = lbh % 2
    t_glob = toff[lb_s, half_s] + rank // 128  # global tile id
    p_slot = rank % 128

    S_all = np.zeros((N_CORES, 128, T_TOTAL * 128), np.float32)
    S_all[core_s, p_slot, t_glob * 128 + dl_s] = w_s

    # dma_gather flat order: flat[i] = idxs[i % 16, i // 16]; i is the
    # message index within the call starting at tile toff[lb, half].
    idx_all = np.zeros((N_CORES, 16, T_TOTAL * 8), np.int16)
    colbase = toff[lb_s, half_s] * 8
    idx_all[core_s, rank % 16, colbase + rank // 16] = idxl_s.astype(np.int16)
    idx_rep = np.tile(idx_all, (1, 8, 1))  # [N_CORES, 128, T*8]

    # real-node mask per core [1, M_PAD]
    real = np.zeros(TOT, np.float32)
    real[pos] = 1.0
    mask = real.reshape(N_CORES, 1, M_PAD)

    # dinv^2 per padded slot, laid out [core, 128, NB] (partition p, block b)
    d2_flat = np.zeros(TOT, np.float32)
    d2_flat[pos] = (dinv * dinv).astype(np.float32)
    dinv2 = d2_flat.reshape(N_CORES, NB, 128).transpose(0, 2, 1).copy()

    # xT padded per core [D, M_PAD]
    x = np.asarray(x, dtype=np.float32)
    xT_pad = np.zeros((D, TOT), np.float32)
    xT_pad[:, pos] = x.T
    xT_own = np.stack([xT_pad[:, c * M_PAD:(c + 1) * M_PAD] for c in range(N_CORES)])

    meta = dict(L_PAD=L_PAD, M_PAD=M_PAD, TOT=TOT, HALF=HALF, NB=NB,
                K_FIX=K_FIX, toff=toff, T_TOTAL=T_TOTAL)
    return meta, S_all, idx_rep, mask, xT_own, dinv2


def _build_bass(meta, weights, repeat=1, ablate=()):
    from concourse import mybir, bacc
    import concourse.tile as tile
    from concourse.masks import make_identity

    L_PAD = meta['L_PAD']; M_PAD = meta['M_PAD']; TOT = meta['TOT']
    HALF = meta['HALF']; NB = meta['NB']
    K_FIX = meta['K_FIX']; toff = meta['toff']; T_TOTAL = meta['T_TOTAL']
    f32 = mybir.dt.float32
    bf16 = mybir.dt.bfloat16
    i16 = mybir.dt.int16
    FCH = [(0, 128), (128, 128), (256, 64)]  # feature chunks of 320

    nc = bacc.Bacc("TRN2", target_bir_lowering=False, debug=False,
                   num_devices=N_CORES, num_swdge_queues=4)

    # ---- IO ----
    S_t = nc.dram_tensor("S_all", [128, T_TOTAL * 128], bf16, kind="ExternalInput")
    idx_t = nc.dram_tensor("idx_all", [128, T_TOTAL * 8], i16, kind="ExternalInput")
    mask_t = nc.dram_tensor("mask", [1, M_PAD], bf16, kind="ExternalInput")
    dinv2_t = nc.dram_tensor("dinv2", [128, NB], f32, kind="ExternalInput")
    xT_t = nc.dram_tensor("xT", [D, M_PAD], bf16, kind="ExternalInput")
    W_t = [nc.dram_tensor(f"W{k}", [D, D], bf16, kind="ExternalInput") for k in (1, 2, 3)]
    brow_t = [nc.dram_tensor(f"brow{k}", [1, D], bf16, kind="ExternalInput") for k in (1, 2, 3)]
    Wf1_t = nc.dram_tensor("Wf1", [320, 256], f32, kind="ExternalInput")
    bf1_t = nc.dram_tensor("bf1c", [128, 2], f32, kind="ExternalInput")
    Wf2_t = nc.dram_tensor("Wf2", [256, 16], f32, kind="ExternalInput")
    bf2_t = nc.dram_tensor("bf2c", [16, 1], f32, kind="ExternalInput")
    Wf3_t = nc.dram_tensor("Wf3", [16, 1], f32, kind="ExternalInput")
    out_t = nc.dram_tensor("out", [1, N_GRAPHS], f32, kind="ExternalOutput")
    bf3_val = float(np.asarray(weights['bf3']).reshape(-1)[0])

    p_own = [nc.dram_tensor(f"p_own{k}", [M_PAD, DP], bf16, kind="Internal")
             for k in range(3)]
    p_full = [nc.dram_tensor(f"p_full{k}", [TOT, DP], bf16, kind="Internal",
                             addr_space="Shared") for k in range(3)]
    pooled_own = nc.dram_tensor("pooled_own", [D, GPC], f32, kind="Internal")
    pooled_all = nc.dram_tensor("pooled_all", [N_CORES, D, GPC], f32,
                                kind="Internal", addr_space="Shared")

    RG = [list(range(N_CORES))]
    KMAXB = int((K_FIX[:, 0] + K_FIX[:, 1]).max())

    with tile.TileContext(nc) as tc:
        with tc.tile_pool(name="persist", bufs=1) as pp, \
             tc.tile_pool(name="gpool", bufs=2) as gp, \
             tc.tile_pool(name="spool", bufs=2) as sp, \
             tc.tile_pool(name="hpool", bufs=2) as hp, \
             tc.tile_pool(name="psum_a", bufs=2, space="PSUM") as pa, \
             tc.tile_pool(name="psum_t", bufs=2, space="PSUM") as pt, \
             tc.tile_pool(name="psum_f", bufs=2, space="PSUM") as pf:

            ident_f = pp.tile([128, 128], f32, tag="ident_f")
            make_identity(nc, ident_f[:])
            ident = pp.tile([128, 128], bf16, tag="ident")
            nc.vector.tensor_copy(ident[:], ident_f[:])
            idx_sb = pp.tile([128, T_TOTAL * 8], i16, tag="idx")
            nc.sync.dma_start(idx_sb[:], idx_t[:, :])
            d2_sb = pp.tile([128, NB], f32, tag="d2")
            nc.sync.dma_start(d2_sb[:], dinv2_t[:, :])
            brow_sb = pp.tile([1, 3 * D], bf16, tag="brow")
            for k in range(3):
                nc.sync.dma_start(brow_sb[:, k * D:(k + 1) * D], brow_t[k][:, :])
            mask_sb = pp.tile([1, M_PAD], bf16, tag="mask")
            nc.sync.dma_start(mask_sb[:], mask_t[:, :])

            # persistent transposed features hT (3 chunks)
            hT = [pp.tile([cl, M_PAD], bf16, tag=f"hT{ci}", name=f"hT{ci}")
                  for ci, (cs, cl) in enumerate(FCH)]
            for ci, (cs, cl) in enumerate(FCH):
                nc.sync.dma_start(hT[ci][:], xT_t[cs:cs + cl, :])

            # persistent own-p (this layer's p for own nodes), node-major
            pown_sb = pp.tile([128, NB * D], bf16, tag="pown")

            # W chunks for the current layer (rhs [128,320] x3), reloaded per layer
            def feature_matmul_phase(layer):
                """p_own[layer] = h @ W[layer] for own nodes (h given by hT)."""
                wl = []
                for ci, (cs, cl) in enumerate(FCH):
                    wt = hp.tile([cl, D], bf16, tag=f"wch{ci}", name=f"wch{ci}", bufs=1)
                    nc.sync.dma_start(wt[:], W_t[layer][cs:cs + cl, :])
                    wl.append(wt)
                for nb in range(NB):
                    ps = pf.tile([128, D], f32, tag="pfeat")
                    for ci, (cs, cl) in enumerate(FCH):
                        nc.tensor.matmul(
                            ps[:],
                            lhsT=hT[ci][:, nb * 128:(nb + 1) * 128],
                            rhs=wl[ci][:],
                            start=(ci == 0), stop=(ci == 2))
                    nc.any.tensor_copy(pown_sb[:, nb * D:(nb + 1) * D], ps[:])
                    nc.sync.dma_start(p_own[layer][nb * 128:(nb + 1) * 128, 0:D],
                                      pown_sb[:, nb * D:(nb + 1) * D])
                if 'ag' not in ablate:
                    nc.gpsimd.collective_compute(
                        "AllGather", mybir.AluOpType.bypass, replica_groups=RG,
                        ins=[p_own[layer][:, :]], outs=[p_full[layer][:, :]])

            qn = [0]

            def agg_phase(layer):
                """hT = relu(Ahat @ p_full[layer] + b) transposed, per dst block."""
                table = p_full[layer]
                for b in range(NB):
                    k0, k1 = int(K_FIX[b, 0]), int(K_FIX[b, 1])
                    ktot = k0 + k1
                    t0 = int(toff[b, 0])
                    g = gp.tile([128, KMAXB, DP], bf16, tag="g")
                    for hh, (kh, th) in enumerate(((k0, int(toff[b, 0])),
                                                   (k1, int(toff[b, 1])))):
                        if kh == 0 or 'gather' in ablate:
                            continue
                        koff = 0 if hh == 0 else k0
                        nc.gpsimd.dma_gather(
                            out_ap=g[:, koff:koff + kh, :],
                            in_ap=table[hh * HALF:(hh + 1) * HALF, :],
                            idxs_ap=idx_sb[:, th * 8:(th + kh) * 8],
                            num_idxs=kh * 128,
                            num_idxs_reg=kh * 128,
                            elem_size=DP,
                            single_packet=False,
                            queue_num=qn[0] % 4)
                        qn[0] += 1
                    s_sb = sp.tile([128, KMAXB * 128], bf16, tag="s")
                    if 'sload' not in ablate:
                        nc.sync.dma_start(s_sb[:, :ktot * 128],
                                          S_t[:, t0 * 128:(t0 + ktot) * 128])
                    ps = pa.tile([128, D], f32, tag="pagg")
                    nc.tensor.matmul(
                        ps[:],
                        lhsT=mask_sb[:, b * 128:(b + 1) * 128],
                        rhs=brow_sb[:, layer * D:(layer + 1) * D],
                        start=True, stop=False)
                    for t in range(ktot):
                        if 'smm' in ablate:
                            break
                        nc.tensor.matmul(
                            ps[:],
                            lhsT=s_sb[:, t * 128:(t + 1) * 128],
                            rhs=g[:, t, 0:D],
                            start=False, stop=(t == ktot - 1))
                    if 'smm' in ablate:
                        nc.tensor.matmul(
                            ps[:], lhsT=s_sb[:, 0:128], rhs=g[:, 0, 0:D],
                            start=False, stop=True)
                    slt = hp.tile([128, D], f32, tag="slt",
                                  name=f"slt_{layer}_{b}")
                    nc.vector.tensor_scalar_mul(slt[:],
                                                pown_sb[:, b * D:(b + 1) * D],
                                                d2_sb[:, b:b + 1])
                    htmp = hp.tile([128, D], bf16, tag="htmp")
                    nc.vector.tensor_tensor(out=htmp[:], in0=ps[:], in1=slt[:],
                                            op=mybir.AluOpType.add)
                    nc.vector.tensor_scalar_max(htmp[:], htmp[:], 0.0)
                    for ci, (cs, cl) in enumerate(FCH):
                        tp = pt.tile([128, 128], bf16, tag="tr")
                        nc.tensor.transpose(tp[:cl, :], htmp[:, cs:cs + cl],
                                            identity=ident[:])
                        nc.any.tensor_copy(hT[ci][:, b * 128:(b + 1) * 128],
                                           tp[:cl, :])

            # ---- network ----
            import contextlib
            loop_ctx = tc.For_i(0, repeat, 1) if repeat > 1 else contextlib.nullcontext()
            with loop_ctx:
                if 'layers' not in ablate:
                    feature_matmul_phase(0)  # p1 = x @ W1
                    agg_phase(0)             # h1
                    feature_matmul_phase(1)  # p2 = h1 @ W2
                    agg_phase(1)             # h2
                    feature_matmul_phase(2)  # p3 = h2 @ W3
                    agg_phase(2)             # h3 (lives in hT)

            # ---- global max pool ----
            for ci, (cs, cl) in enumerate(FCH):
                gt = hp.tile([cl, GPC], f32, tag=f"gt{ci}", name=f"gt{ci}", bufs=1)
                for j in range(GPC):
                    nc.vector.reduce_max(
                        gt[:, j:j + 1], hT[ci][:, j * L_PAD:(j + 1) * L_PAD],
                        axis=mybir.AxisListType.X)
                nc.sync.dma_start(pooled_own[cs:cs + cl, :], gt[:])
            nc.gpsimd.collective_compute(
                "AllGather", mybir.AluOpType.bypass, replica_groups=RG,
                ins=[pooled_own[:, :]], outs=[pooled_all[:, :, :]])

            # gT_full chunks [cl, 64]
            gT = []
            for ci, (cs, cl) in enumerate(FCH):
                gtile = hp.tile([cl, N_GRAPHS], f32, tag=f"gTf{ci}", name=f"gTf{ci}", bufs=1)
                for cc in range(N_CORES):
                    nc.sync.dma_start(gtile[:, cc * GPC:(cc + 1) * GPC],
                                      pooled_all[cc, cs:cs + cl, :])
                gT.append(gtile)

            # ---- MLP head (transposed): z1T[256,64] ----
            wf1 = []
            for mi in range(2):
                for ci, (cs, cl) in enumerate(FCH):
                    t = hp.tile([cl, 128], f32, tag=f"wf1_{mi}_{ci}", name=f"wf1_{mi}_{ci}", bufs=1)
                    nc.sync.dma_start(t[:], Wf1_t[cs:cs + cl, mi * 128:(mi + 1) * 128])
                    wf1.append(t)
            bf1sb = hp.tile([128, 2], f32, tag="bf1", bufs=1)
            nc.sync.dma_start(bf1sb[:], bf1_t[:, :])
            h1T = []
            for mi in range(2):
                ps = pf.tile([128, N_GRAPHS], f32, tag="pfeat")
                for ci in range(3):
                    nc.tensor.matmul(ps[:], lhsT=wf1[mi * 3 + ci][:],
                                     rhs=gT[ci][:],
                                     start=(ci == 0), stop=(ci == 2))
                h = hp.tile([128, N_GRAPHS], f32, tag=f"h1T{mi}", name=f"h1T{mi}", bufs=1)
                nc.vector.tensor_scalar(h[:], ps[:],
                                        bf1sb[:, mi:mi + 1], 0.0,
                                        op0=mybir.AluOpType.add,
                                        op1=mybir.AluOpType.max)
                h1T.append(h)
            # z2T [16, 64]
            wf2 = []
            for mi in range(2):
                t = hp.tile([128, 16], f32, tag=f"wf2_{mi}", name=f"wf2_{mi}", bufs=1)
                nc.sync.dma_start(t[:], Wf2_t[mi * 128:(mi + 1) * 128, :])
                wf2.append(t)
            bf2sb = hp.tile([16, 1], f32, tag="bf2", bufs=1)
            nc.sync.dma_start(bf2sb[:], bf2_t[:, :])
            ps2 = pf.tile([16, N_GRAPHS], f32, tag="pfeat")
            for mi in range(2):
                nc.tensor.matmul(ps2[:], lhsT=wf2[mi][:],
                                 rhs=h1T[mi][:],
                                 start=(mi == 0), stop=(mi == 1))
            h2T = hp.tile([16, N_GRAPHS], f32, tag="h2T", bufs=1)
            nc.vector.tensor_scalar(h2T[:], ps2[:], bf2sb[:, 0:1], 0.0,
                                    op0=mybir.AluOpType.add,
                                    op1=mybir.AluOpType.max)
            # z3 [1, 64]
            wf3 = hp.tile([16, 1], f32, tag="wf3", bufs=1)
            nc.sync.dma_start(wf3[:], Wf3_t[:, :])
            ps3 = pf.tile([1, N_GRAPHS], f32, tag="pfeat")
            nc.tensor.matmul(ps3[:], lhsT=wf3[:],
                             rhs=h2T[:], start=True, stop=True)
            osb = hp.tile([1, N_GRAPHS], f32, tag="osb", bufs=1)
            nc.vector.tensor_scalar(osb[:], ps3[:], bf3_val, None,
                                    op0=mybir.AluOpType.add)
            nc.sync.dma_start(out_t[:, :], osb[:])

    nc.compile()
    return nc


def _make_runner(nc, in_maps):
    """Build a reusable jitted SPMD executor for `nc` (axon/PJRT path).

    Returns (run_fn, out_names, out_avals): run_fn() executes once and
    returns the list of per-core result dicts.
    """
    import jax
    import numpy as np
    from jax.experimental.shard_map import shard_map
    from jax.sharding import Mesh, NamedSharding, PartitionSpec
    from concourse import bass2jax, mybir

    bass2jax.install_neuronx_cc_hook()
    n_cores = len(in_maps)
    partition_name = nc.partition_id_tensor.name if nc.partition_id_tensor else None
    in_names, out_names, out_avals, zero_outs = [], [], [], []
    for alloc in nc.m.functions[0].allocations:
        if not isinstance(alloc, mybir.MemoryLocationSet):
            continue
        name = alloc.memorylocations[0].name
        if alloc.kind == "ExternalInput":
            if name != partition_name:
                in_names.append(name)
        elif alloc.kind == "ExternalOutput":
            shape = tuple(alloc.tensor_shape)
            dtype = mybir.dt.np(alloc.dtype)
            out_names.append(name)
            out_avals.append(jax.core.ShapedArray(shape, dtype))
            zero_outs.append(np.zeros(shape, dtype))
    n_params = len(in_names)
    n_outs = len(out_avals)
    all_in_names = list(in_names) + list(out_names)
    if partition_name is not None:
        all_in_names.append(partition_name)
    donate = tuple(range(n_params, n_params + n_outs))

    def _body(*args):
        operands = list(args)
        if partition_name is not None:
            operands.append(bass2jax.partition_id_tensor())
        outs = bass2jax._bass_exec_p.bind(
            *operands,
            out_avals=tuple(out_avals),
            in_names=tuple(all_in_names),
            out_names=tuple(out_names),
            lowering_input_output_aliases=(),
            sim_require_finite=True,
            sim_require_nnan=True,
            nc=nc,
        )
        return tuple(outs)

    devices = jax.devices()[:n_cores]
    mesh = Mesh(np.asarray(devices), ("core",))
    in_specs = (PartitionSpec("core"),) * (n_params + n_outs)
    out_specs = (PartitionSpec("core"),) * len(out_names)
    sharded = jax.jit(
        shard_map(_body, mesh=mesh, in_specs=in_specs, out_specs=out_specs,
                  check_rep=False),
        donate_argnums=donate, keep_unused=True)
    sh = NamedSharding(mesh, PartitionSpec("core"))
    concat_in = [
        jax.device_put(
            np.concatenate([np.asarray(in_maps[c][nm]) for c in range(n_cores)],
                           axis=0), sh)
        for nm in in_names
    ]

    def run_fn():
        zeros = [np.zeros((n_cores * z.shape[0], *z.shape[1:]), z.dtype)
                 for z in zero_outs]
        out_arrs = sharded(*concat_in, *zeros)
        out_arrs = [np.asarray(o) for o in out_arrs]
        return [
            {nm: out_arrs[i].reshape(n_cores, *out_avals[i].shape)[c]
             for i, nm in enumerate(out_names)}
            for c in range(n_cores)
        ]

    return run_fn, out_names, out_avals


def prepare(inputs, repeat=1, ablate=()):
    """Preprocess + build + compile; returns a reusable run_fn."""
    meta, S_all, idx_rep, mask, xT_own, dinv2 = _preprocess(
        inputs['x'], inputs['edge_index'], inputs['batch'])
    nc = _build_bass(meta, inputs, repeat=repeat, ablate=ablate)
    in_maps = _make_in_maps(inputs, S_all, idx_rep, mask, xT_own, dinv2)
    run_fn, _, _ = _make_runner(nc, in_maps)
    return run_fn


def _make_in_maps(inputs, S_all, idx_rep, mask, xT_own, dinv2):
    import ml_dtypes
    bf = ml_dtypes.bfloat16
    in_maps = []
    for c in range(N_CORES):
        m = {
            "S_all": S_all[c].astype(bf),
            "idx_all": idx_rep[c],
            "mask": mask[c].astype(bf),
            "dinv2": np.ascontiguousarray(dinv2[c]),
            "xT": np.ascontiguousarray(xT_own[c]).astype(bf),
            "Wf1": np.asarray(inputs['Wf1'], np.float32),
            "bf1c": np.ascontiguousarray(
                np.asarray(inputs['bf1'], np.float32).reshape(2, 128).T),
            "Wf2": np.asarray(inputs['Wf2'], np.float32),
            "bf2c": np.asarray(inputs['bf2'], np.float32).reshape(16, 1),
            "Wf3": np.asarray(inputs['Wf3'], np.float32),
        }
        for k in (1, 2, 3):
            m[f"W{k}"] = np.asarray(inputs[f'W{k}'], np.float32).astype(bf)
            m[f"brow{k}"] = np.asarray(inputs[f'b{k}'], np.float32).reshape(1, D).astype(bf)
        in_maps.append(m)
    return in_maps


def kernel(**inputs):
    meta, S_all, idx_rep, mask, xT_own, dinv2 = _preprocess(
        inputs['x'], inputs['edge_index'], inputs['batch'])
    nc = _build_bass(meta, inputs)
    in_maps = _make_in_maps(inputs, S_all, idx_rep, mask, xT_own, dinv2)
    from concourse.bass_utils import run_bass_kernel_spmd
    res = run_bass_kernel_spmd(nc, in_maps, core_ids=list(range(N_CORES)),
                               trace=False)
    out = np.asarray(res.results[0]["out"]).reshape(1, N_GRAPHS)
    return out.T.copy()


# revision 20
# speedup vs baseline: 1.6943x; 1.5747x over previous
"""GCN (3x GCNConv + global max pool + MLP) on 8 Trainium2 NeuronCores.

Strategy (data-parallel over graphs, per sharding hint):
 - Nodes laid out graph-padded: each graph gets a fixed slot of L_PAD columns;
   core c owns graphs [8c, 8c+8) -> M_PAD = 8*L_PAD padded node slots.
 - bf16 data path: p table, gathers, S matrices, hT all bf16 (PSUM math fp32).
   Table rows padded to 384 cols so gather elem bytes (768) is a 256 multiple.
 - Per layer: p = h @ W computed for own nodes (kept in SBUF + written to
   DRAM), AllGather -> replicated p table, per-edge gather of p[src] rows via
   dma_gather (edges sharded by dst core, grouped by 128-dst block),
   aggregation as PE matmuls with host-precomputed weighted one-hot selection
   matrices S (norm folded in, bias via a rank-1 mask-matmul), self-loop term
   from the SBUF-resident p_own, relu on evacuation, transpose to
   feature-major for the next layer's lhsT.
 - Pooling: per-graph column-slices reduce_max (pad cols are exactly 0, relu
   output >= 0, so padding never changes the max). Pooled vectors AllGathered,
   MLP head computed redundantly on every core.
"""
import os
import sys
import numpy as np

for _p in ('/opt/trn_rl_repo', '/root/.axon_site/_ro/trn_rl_repo'):
    if os.path.isdir(_p) and _p not in sys.path:
        sys.path.insert(0, _p)

N_CORES = 8
N_NODES = 50000
D = 320
DP = 384  # padded table row (bf16 -> 768B, multiple of 256)
DP8 = 512  # padded table row (fp8 -> 512B, multiple of 256)
N_GRAPHS = 64
GPC = N_GRAPHS // N_CORES  # graphs per core
# per-layer gather-table dtype: 'f8' (e4m3, 512B rows) or 'bf16' (768B rows)
TABLE_DTS = ('f8', 'f8', 'f8')


def _preprocess(x, edge_index, batch):
    """Build per-core gather indices, selection matrices and layouts."""
    batch = np.asarray(batch).astype(np.int64)
    src = np.asarray(edge_index[0]).astype(np.int64)
    dst = np.asarray(edge_index[1]).astype(np.int64)
    counts = np.bincount(batch, minlength=N_GRAPHS)
    L_PAD = max(896, int(-(-counts.max() // 128)) * 128)
    M_PAD = GPC * L_PAD
    TOT = N_CORES * M_PAD
    HALF = TOT // 2
    assert HALF <= 32767, (L_PAD, HALF)
    NB = M_PAD // 128  # dst blocks per core

    gstart = np.zeros(N_GRAPHS, np.int64)
    gstart[1:] = np.cumsum(counts)[:-1]
    n_ar = np.arange(N_NODES, dtype=np.int64)
    # within-graph relabeling: deal nodes (sorted by in-degree desc) round-robin
    # into the graph's L_PAD/128 blocks so per-block in-degree is balanced,
    # which minimizes the uniform per-(block,half) tile count K_FIX.
    NBLK = L_PAD // 128
    indeg = np.bincount(dst, minlength=N_NODES) + 1
    order_bal = np.lexsort((-indeg, batch))
    r = n_ar - gstart[batch[order_bal]]
    posg = (r % NBLK) * 128 + r // NBLK
    pos_in_graph = np.empty(N_NODES, np.int64)
    pos_in_graph[order_bal] = posg
    pos = (batch // GPC) * M_PAD + (batch % GPC) * L_PAD + pos_in_graph

    deg = np.bincount(dst, minlength=N_NODES).astype(np.float64) + 1.0
    dinv = 1.0 / np.sqrt(deg)

    # self-loops are applied locally (h += dinv^2 * p_own), not gathered
    ms = src
    mt = dst
    w = (dinv[src] * dinv[dst]).astype(np.float32)

    ms_pos = pos[ms]
    mt_pos = pos[mt]
    core = mt_pos // M_PAD
    lb = (mt_pos % M_PAD) // 128
    dl = mt_pos % 128
    half = ms_pos // HALF
    idxl = (ms_pos % HALF).astype(np.int64)

    key = (core * NB + lb) * 2 + half
    order = np.argsort(key, kind='stable')
    key_s = key[order]
    idxl_s = idxl[order]
    dl_s = dl[order]
    w_s = w[order]

    nkeys = N_CORES * NB * 2
    kcounts = np.bincount(key_s, minlength=nkeys).reshape(N_CORES, NB, 2)
    k_req = -(-kcounts // 128)  # ceil
    K_FIX = k_req.max(axis=0)  # [NB, 2] uniform across cores
    toff = np.zeros((NB, 2), np.int64)
    flat_k = K_FIX.reshape(-1)
    toff.reshape(-1)[1:] = np.cumsum(flat_k)[:-1]
    T_TOTAL = int(flat_k.sum())

    # rank of each message within its (core, lb, half) group
    kstart = np.zeros(nkeys, np.int64)
    kstart[1:] = np.cumsum(np.bincount(key_s, minlength=nkeys))[:-1]
    rank = np.arange(len(key_s)) - kstart[key_s]

    core_s = key_s // (NB * 2)
    lbh = key_s % (NB * 2)
    lb_s = lbh // 2
    half_s = lbh % 2
    t_glob = toff[lb_s, half_s] + rank // 128  # global tile id
    p_slot = rank % 128

    S_all = np.zeros((N_CORES, 128, T_TOTAL * 128), np.float32)
    S_all[core_s, p_slot, t_glob * 128 + dl_s] = w_s

    # dma_gather flat order: flat[i] = idxs[i % 16, i // 16]; i is the
    # message index within the call starting at tile toff[lb, half].
    idx_all = np.zeros((N_CORES, 16, T_TOTAL * 8), np.int16)
    colbase = toff[lb_s, half_s] * 8
    idx_all[core_s, rank % 16, colbase + rank // 16] = idxl_s.astype(np.int16)
    idx_rep = np.tile(idx_all, (1, 8, 1))  # [N_CORES, 128, T*8]

    # real-node mask per core [1, M_PAD]
    real = np.zeros(TOT, np.float32)
    real[pos] = 1.0
    mask = real.reshape(N_CORES, 1, M_PAD)

    # dinv^2 per padded slot, laid out [core, 128, NB] (partition p, block b)
    d2_flat = np.zeros(TOT, np.float32)
    d2_flat[pos] = (dinv * dinv).astype(np.float32)
    dinv2 = d2_flat.reshape(N_CORES, NB, 128).transpose(0, 2, 1).copy()

    # xT padded per core [D, M_PAD]
    x = np.asarray(x, dtype=np.float32)
    xT_pad = np.zeros((D, TOT), np.float32)
    xT_pad[:, pos] = x.T
    xT_own = np.stack([xT_pad[:, c * M_PAD:(c + 1) * M_PAD] for c in range(N_CORES)])

    meta = dict(L_PAD=L_PAD, M_PAD=M_PAD, TOT=TOT, HALF=HALF, NB=NB,
                K_FIX=K_FIX, toff=toff, T_TOTAL=T_TOTAL)
    return meta, S_all, idx_rep, mask, xT_own, dinv2


def _build_bass(meta, weights, repeat=1, ablate=()):
    from concourse import mybir, bacc
    import concourse.tile as tile
    from concourse.masks import make_identity

    L_PAD = meta['L_PAD']; M_PAD = meta['M_PAD']; TOT = meta['TOT']
    HALF = meta['HALF']; NB = meta['NB']
    K_FIX = meta['K_FIX']; toff = meta['toff']; T_TOTAL = meta['T_TOTAL']
    f32 = mybir.dt.float32
    bf16 = mybir.dt.bfloat16
    f8 = mybir.dt.float8e4
    i16 = mybir.dt.int16
    TDT = [f8 if t == 'f8' else bf16 for t in TABLE_DTS]
    TDP = [DP8 if t == 'f8' else DP for t in TABLE_DTS]
    FCH = [(0, 128), (128, 128), (256, 64)]  # feature chunks of 320

    nc = bacc.Bacc("TRN2", target_bir_lowering=False, debug=False,
                   num_devices=N_CORES, num_swdge_queues=4)

    # ---- IO ----
    S_t = nc.dram_tensor("S_all", [128, T_TOTAL * 128], bf16, kind="ExternalInput")
    idx_t = nc.dram_tensor("idx_all", [128, T_TOTAL * 8], i16, kind="ExternalInput")
    mask_t = nc.dram_tensor("mask", [1, M_PAD], bf16, kind="ExternalInput")
    dinv2_t = nc.dram_tensor("dinv2", [128, NB], f32, kind="ExternalInput")
    xT_t = nc.dram_tensor("xT", [D, M_PAD], bf16, kind="ExternalInput")
    W_t = [nc.dram_tensor(f"W{k}", [D, D], bf16, kind="ExternalInput") for k in (1, 2, 3)]
    brow_t = [nc.dram_tensor(f"brow{k}", [1, D], bf16, kind="ExternalInput") for k in (1, 2, 3)]
    Wf1_t = nc.dram_tensor("Wf1", [320, 256], f32, kind="ExternalInput")
    bf1_t = nc.dram_tensor("bf1c", [128, 2], f32, kind="ExternalInput")
    Wf2_t = nc.dram_tensor("Wf2", [256, 16], f32, kind="ExternalInput")
    bf2_t = nc.dram_tensor("bf2c", [16, 1], f32, kind="ExternalInput")
    Wf3_t = nc.dram_tensor("Wf3", [16, 1], f32, kind="ExternalInput")
    out_t = nc.dram_tensor("out", [1, N_GRAPHS], f32, kind="ExternalOutput")
    bf3_val = float(np.asarray(weights['bf3']).reshape(-1)[0])

    p_own = [nc.dram_tensor(f"p_own{k}", [M_PAD, TDP[k]], TDT[k], kind="Internal")
             for k in range(3)]
    p_full = [nc.dram_tensor(f"p_full{k}", [TOT, TDP[k]], TDT[k], kind="Internal",
                             addr_space="Shared") for k in range(3)]
    pooled_own = nc.dram_tensor("pooled_own", [D, GPC], f32, kind="Internal")
    pooled_all = nc.dram_tensor("pooled_all", [N_CORES, D, GPC], f32,
                                kind="Internal", addr_space="Shared")

    RG = [list(range(N_CORES))]
    KMAXB = int((K_FIX[:, 0] + K_FIX[:, 1]).max())

    with tile.TileContext(nc) as tc:
        with tc.tile_pool(name="persist", bufs=1) as pp, \
             tc.tile_pool(name="gpool", bufs=2) as gp, \
             tc.tile_pool(name="spool", bufs=2) as sp, \
             tc.tile_pool(name="hpool", bufs=2) as hp, \
             tc.tile_pool(name="psum_a", bufs=2, space="PSUM") as pa, \
             tc.tile_pool(name="psum_t", bufs=2, space="PSUM") as pt, \
             tc.tile_pool(name="psum_f", bufs=2, space="PSUM") as pf:

            ident_f = pp.tile([128, 128], f32, tag="ident_f")
            make_identity(nc, ident_f[:])
            ident = pp.tile([128, 128], bf16, tag="ident")
            nc.vector.tensor_copy(ident[:], ident_f[:])
            idx_sb = pp.tile([128, T_TOTAL * 8], i16, tag="idx")
            nc.sync.dma_start(idx_sb[:], idx_t[:, :])
            d2_sb = pp.tile([128, NB], f32, tag="d2")
            nc.sync.dma_start(d2_sb[:], dinv2_t[:, :])
            brow_sb = pp.tile([1, 3 * D], bf16, tag="brow")
            for k in range(3):
                nc.sync.dma_start(brow_sb[:, k * D:(k + 1) * D], brow_t[k][:, :])
            mask_sb = pp.tile([1, M_PAD], bf16, tag="mask")
            nc.sync.dma_start(mask_sb[:], mask_t[:, :])

            # persistent transposed features hT (3 chunks)
            hT = [pp.tile([cl, M_PAD], bf16, tag=f"hT{ci}", name=f"hT{ci}")
                  for ci, (cs, cl) in enumerate(FCH)]
            for ci, (cs, cl) in enumerate(FCH):
                nc.sync.dma_start(hT[ci][:], xT_t[cs:cs + cl, :])

            # persistent own-p (this layer's p for own nodes), node-major
            pown_sb = pp.tile([128, NB * D], bf16, tag="pown")

            # W chunks for the current layer (rhs [128,320] x3), reloaded per layer
            def feature_matmul_phase(layer):
                """p_own[layer] = h @ W[layer] for own nodes (h given by hT)."""
                wl = []
                for ci, (cs, cl) in enumerate(FCH):
                    wt = hp.tile([cl, D], bf16, tag=f"wch{ci}", name=f"wch{ci}", bufs=1)
                    nc.sync.dma_start(wt[:], W_t[layer][cs:cs + cl, :])
                    wl.append(wt)
                for nb in range(NB):
                    ps = pf.tile([128, D], f32, tag="pfeat")
                    for ci, (cs, cl) in enumerate(FCH):
                        nc.tensor.matmul(
                            ps[:],
                            lhsT=hT[ci][:, nb * 128:(nb + 1) * 128],
                            rhs=wl[ci][:],
                            start=(ci == 0), stop=(ci == 2))
                    nc.any.tensor_copy(pown_sb[:, nb * D:(nb + 1) * D], ps[:])
                    if TDT[layer] == bf16:
                        nc.sync.dma_start(
                            p_own[layer][nb * 128:(nb + 1) * 128, 0:D],
                            pown_sb[:, nb * D:(nb + 1) * D])
                    else:
                        pq = hp.tile([128, D], f8, tag="pq")
                        nc.vector.tensor_copy(pq[:], ps[:])
                        nc.sync.dma_start(
                            p_own[layer][nb * 128:(nb + 1) * 128, 0:D], pq[:])
                if 'ag' not in ablate:
                    nc.gpsimd.collective_compute(
                        "AllGather", mybir.AluOpType.bypass, replica_groups=RG,
                        ins=[p_own[layer][:, :]], outs=[p_full[layer][:, :]])

            qn = [0]

            def agg_phase(layer):
                """hT = relu(Ahat @ p_full[layer] + b) transposed, per dst block."""
                table = p_full[layer]
                ldt, ldp = TDT[layer], TDP[layer]
                for b in range(NB):
                    k0, k1 = int(K_FIX[b, 0]), int(K_FIX[b, 1])
                    ktot = k0 + k1
                    t0 = int(toff[b, 0])
                    g = gp.tile([128, KMAXB, ldp], ldt, tag="g")
                    if 'gather' in ablate:
                        nc.vector.memset(g[:, 0, 0:8], 0.125)
                    for hh, (kh, th) in enumerate(((k0, int(toff[b, 0])),
                                                   (k1, int(toff[b, 1])))):
                        if kh == 0 or 'gather' in ablate:
                            continue
                        koff = 0 if hh == 0 else k0
                        nc.gpsimd.dma_gather(
                            out_ap=g[:, koff:koff + kh, :],
                            in_ap=table[hh * HALF:(hh + 1) * HALF, :],
                            idxs_ap=idx_sb[:, th * 8:(th + kh) * 8],
                            num_idxs=kh * 128,
                            num_idxs_reg=kh * 128,
                            elem_size=ldp,
                            single_packet=False,
                            queue_num=qn[0] % 4)
                        qn[0] += 1
                    s_sb = sp.tile([128, KMAXB * 128], bf16, tag="s")
                    if 'sload' not in ablate:
                        nc.sync.dma_start(s_sb[:, :ktot * 128],
                                          S_t[:, t0 * 128:(t0 + ktot) * 128])
                    else:
                        nc.vector.memset(s_sb[:, 0:8], 0.0)
                    ps = pa.tile([128, D], f32, tag="pagg")
                    nc.tensor.matmul(
                        ps[:],
                        lhsT=mask_sb[:, b * 128:(b + 1) * 128],
                        rhs=brow_sb[:, layer * D:(layer + 1) * D],
                        start=True, stop=False)
                    for t in range(ktot):
                        if 'smm' in ablate:
                            break
                        nc.tensor.matmul(
                            ps[:],
                            lhsT=s_sb[:, t * 128:(t + 1) * 128],
                            rhs=g[:, t, 0:D],
                            start=False, stop=(t == ktot - 1))
                    if 'smm' in ablate:
                        nc.tensor.matmul(
                            ps[:], lhsT=s_sb[:, 0:128], rhs=g[:, 0, 0:D],
                            start=False, stop=True)
                    slt = hp.tile([128, D], f32, tag="slt",
                                  name=f"slt_{layer}_{b}")
                    nc.vector.tensor_scalar_mul(slt[:],
                                                pown_sb[:, b * D:(b + 1) * D],
                                                d2_sb[:, b:b + 1])
                    htmp = hp.tile([128, D], bf16, tag="htmp")
                    nc.vector.tensor_tensor(out=htmp[:], in0=ps[:], in1=slt[:],
                                            op=mybir.AluOpType.add)
                    nc.vector.tensor_scalar_max(htmp[:], htmp[:], 0.0)
                    for ci, (cs, cl) in enumerate(FCH):
                        tp = pt.tile([128, 128], bf16, tag="tr")
                        nc.tensor.transpose(tp[:cl, :], htmp[:, cs:cs + cl],
                                            identity=ident[:])
                        nc.any.tensor_copy(hT[ci][:, b * 128:(b + 1) * 128],
                                           tp[:cl, :])

            # ---- network ----
            import contextlib
            loop_ctx = tc.For_i(0, repeat, 1) if repeat > 1 else contextlib.nullcontext()
            with loop_ctx:
                if 'layers' not in ablate:
                    feature_matmul_phase(0)  # p1 = x @ W1
                    agg_phase(0)             # h1
                    feature_matmul_phase(1)  # p2 = h1 @ W2
                    agg_phase(1)             # h2
                    feature_matmul_phase(2)  # p3 = h2 @ W3
                    agg_phase(2)             # h3 (lives in hT)

            # ---- global max pool ----
            for ci, (cs, cl) in enumerate(FCH):
                gt = hp.tile([cl, GPC], f32, tag=f"gt{ci}", name=f"gt{ci}", bufs=1)
                for j in range(GPC):
                    nc.vector.reduce_max(
                        gt[:, j:j + 1], hT[ci][:, j * L_PAD:(j + 1) * L_PAD],
                        axis=mybir.AxisListType.X)
                nc.sync.dma_start(pooled_own[cs:cs + cl, :], gt[:])
            nc.gpsimd.collective_compute(
                "AllGather", mybir.AluOpType.bypass, replica_groups=RG,
                ins=[pooled_own[:, :]], outs=[pooled_all[:, :, :]])

            # gT_full chunks [cl, 64]
            gT = []
            for ci, (cs, cl) in enumerate(FCH):
                gtile = hp.tile([cl, N_GRAPHS], f32, tag=f"gTf{ci}", name=f"gTf{ci}", bufs=1)
                for cc in range(N_CORES):
                    nc.sync.dma_start(gtile[:, cc * GPC:(cc + 1) * GPC],
                                      pooled_all[cc, cs:cs + cl, :])
                gT.append(gtile)

            # ---- MLP head (transposed): z1T[256,64] ----
            wf1 = []
            for mi in range(2):
                for ci, (cs, cl) in enumerate(FCH):
                    t = hp.tile([cl, 128], f32, tag=f"wf1_{mi}_{ci}", name=f"wf1_{mi}_{ci}", bufs=1)
                    nc.sync.dma_start(t[:], Wf1_t[cs:cs + cl, mi * 128:(mi + 1) * 128])
                    wf1.append(t)
            bf1sb = hp.tile([128, 2], f32, tag="bf1", bufs=1)
            nc.sync.dma_start(bf1sb[:], bf1_t[:, :])
            h1T = []
            for mi in range(2):
                ps = pf.tile([128, N_GRAPHS], f32, tag="pfeat")
                for ci in range(3):
                    nc.tensor.matmul(ps[:], lhsT=wf1[mi * 3 + ci][:],
                                     rhs=gT[ci][:],
                                     start=(ci == 0), stop=(ci == 2))
                h = hp.tile([128, N_GRAPHS], f32, tag=f"h1T{mi}", name=f"h1T{mi}", bufs=1)
                nc.vector.tensor_scalar(h[:], ps[:],
                                        bf1sb[:, mi:mi + 1], 0.0,
                                        op0=mybir.AluOpType.add,
                                        op1=mybir.AluOpType.max)
                h1T.append(h)
            # z2T [16, 64]
            wf2 = []
            for mi in range(2):
                t = hp.tile([128, 16], f32, tag=f"wf2_{mi}", name=f"wf2_{mi}", bufs=1)
                nc.sync.dma_start(t[:], Wf2_t[mi * 128:(mi + 1) * 128, :])
                wf2.append(t)
            bf2sb = hp.tile([16, 1], f32, tag="bf2", bufs=1)
            nc.sync.dma_start(bf2sb[:], bf2_t[:, :])
            ps2 = pf.tile([16, N_GRAPHS], f32, tag="pfeat")
            for mi in range(2):
                nc.tensor.matmul(ps2[:], lhsT=wf2[mi][:],
                                 rhs=h1T[mi][:],
                                 start=(mi == 0), stop=(mi == 1))
            h2T = hp.tile([16, N_GRAPHS], f32, tag="h2T", bufs=1)
            nc.vector.tensor_scalar(h2T[:], ps2[:], bf2sb[:, 0:1], 0.0,
                                    op0=mybir.AluOpType.add,
                                    op1=mybir.AluOpType.max)
            # z3 [1, 64]
            wf3 = hp.tile([16, 1], f32, tag="wf3", bufs=1)
            nc.sync.dma_start(wf3[:], Wf3_t[:, :])
            ps3 = pf.tile([1, N_GRAPHS], f32, tag="pfeat")
            nc.tensor.matmul(ps3[:], lhsT=wf3[:],
                             rhs=h2T[:], start=True, stop=True)
            osb = hp.tile([1, N_GRAPHS], f32, tag="osb", bufs=1)
            nc.vector.tensor_scalar(osb[:], ps3[:], bf3_val, None,
                                    op0=mybir.AluOpType.add)
            nc.sync.dma_start(out_t[:, :], osb[:])

    nc.compile()
    return nc


def _make_runner(nc, in_maps):
    """Build a reusable jitted SPMD executor for `nc` (axon/PJRT path).

    Returns (run_fn, out_names, out_avals): run_fn() executes once and
    returns the list of per-core result dicts.
    """
    import jax
    import numpy as np
    from jax.experimental.shard_map import shard_map
    from jax.sharding import Mesh, NamedSharding, PartitionSpec
    from concourse import bass2jax, mybir

    bass2jax.install_neuronx_cc_hook()
    n_cores = len(in_maps)
    partition_name = nc.partition_id_tensor.name if nc.partition_id_tensor else None
    in_names, out_names, out_avals, zero_outs = [], [], [], []
    for alloc in nc.m.functions[0].allocations:
        if not isinstance(alloc, mybir.MemoryLocationSet):
            continue
        name = alloc.memorylocations[0].name
        if alloc.kind == "ExternalInput":
            if name != partition_name:
                in_names.append(name)
        elif alloc.kind == "ExternalOutput":
            shape = tuple(alloc.tensor_shape)
            dtype = mybir.dt.np(alloc.dtype)
            out_names.append(name)
            out_avals.append(jax.core.ShapedArray(shape, dtype))
            zero_outs.append(np.zeros(shape, dtype))
    n_params = len(in_names)
    n_outs = len(out_avals)
    all_in_names = list(in_names) + list(out_names)
    if partition_name is not None:
        all_in_names.append(partition_name)
    donate = tuple(range(n_params, n_params + n_outs))

    def _body(*args):
        operands = list(args)
        if partition_name is not None:
            operands.append(bass2jax.partition_id_tensor())
        outs = bass2jax._bass_exec_p.bind(
            *operands,
            out_avals=tuple(out_avals),
            in_names=tuple(all_in_names),
            out_names=tuple(out_names),
            lowering_input_output_aliases=(),
            sim_require_finite=True,
            sim_require_nnan=True,
            nc=nc,
        )
        return tuple(outs)

    devices = jax.devices()[:n_cores]
    mesh = Mesh(np.asarray(devices), ("core",))
    in_specs = (PartitionSpec("core"),) * (n_params + n_outs)
    out_specs = (PartitionSpec("core"),) * len(out_names)
    sharded = jax.jit(
        shard_map(_body, mesh=mesh, in_specs=in_specs, out_specs=out_specs,
                  check_rep=False),
        donate_argnums=donate, keep_unused=True)
    sh = NamedSharding(mesh, PartitionSpec("core"))
    concat_in = [
        jax.device_put(
            np.concatenate([np.asarray(in_maps[c][nm]) for c in range(n_cores)],
                           axis=0), sh)
        for nm in in_names
    ]

    def run_fn():
        zeros = [np.zeros((n_cores * z.shape[0], *z.shape[1:]), z.dtype)
                 for z in zero_outs]
        out_arrs = sharded(*concat_in, *zeros)
        out_arrs = [np.asarray(o) for o in out_arrs]
        return [
            {nm: out_arrs[i].reshape(n_cores, *out_avals[i].shape)[c]
             for i, nm in enumerate(out_names)}
            for c in range(n_cores)
        ]

    return run_fn, out_names, out_avals


def prepare(inputs, repeat=1, ablate=()):
    """Preprocess + build + compile; returns a reusable run_fn."""
    meta, S_all, idx_rep, mask, xT_own, dinv2 = _preprocess(
        inputs['x'], inputs['edge_index'], inputs['batch'])
    nc = _build_bass(meta, inputs, repeat=repeat, ablate=ablate)
    in_maps = _make_in_maps(inputs, S_all, idx_rep, mask, xT_own, dinv2)
    run_fn, _, _ = _make_runner(nc, in_maps)
    return run_fn


def _make_in_maps(inputs, S_all, idx_rep, mask, xT_own, dinv2):
    import ml_dtypes
    bf = ml_dtypes.bfloat16
    in_maps = []
    for c in range(N_CORES):
        m = {
            "S_all": S_all[c].astype(bf),
            "idx_all": idx_rep[c],
            "mask": mask[c].astype(bf),
            "dinv2": np.ascontiguousarray(dinv2[c]),
            "xT": np.ascontiguousarray(xT_own[c]).astype(bf),
            "Wf1": np.asarray(inputs['Wf1'], np.float32),
            "bf1c": np.ascontiguousarray(
                np.asarray(inputs['bf1'], np.float32).reshape(2, 128).T),
            "Wf2": np.asarray(inputs['Wf2'], np.float32),
            "bf2c": np.asarray(inputs['bf2'], np.float32).reshape(16, 1),
            "Wf3": np.asarray(inputs['Wf3'], np.float32),
        }
        for k in (1, 2, 3):
            m[f"W{k}"] = np.asarray(inputs[f'W{k}'], np.float32).astype(bf)
            m[f"brow{k}"] = np.asarray(inputs[f'b{k}'], np.float32).reshape(1, D).astype(bf)
        in_maps.append(m)
    return in_maps


def kernel(**inputs):
    meta, S_all, idx_rep, mask, xT_own, dinv2 = _preprocess(
        inputs['x'], inputs['edge_index'], inputs['batch'])
    nc = _build_bass(meta, inputs)
    in_maps = _make_in_maps(inputs, S_all, idx_rep, mask, xT_own, dinv2)
    from concourse.bass_utils import run_bass_kernel_spmd
    res = run_bass_kernel_spmd(nc, in_maps, core_ids=list(range(N_CORES)),
                               trace=False)
    out = np.asarray(res.results[0]["out"]).reshape(1, N_GRAPHS)
    return out.T.copy()


# revision 23
# speedup vs baseline: 1.8483x; 1.0909x over previous
"""GCN (3x GCNConv + global max pool + MLP) on 8 Trainium2 NeuronCores.

Strategy (data-parallel over graphs, per sharding hint):
 - Nodes laid out graph-padded: each graph gets a fixed slot of L_PAD columns;
   core c owns graphs [8c, 8c+8) -> M_PAD = 8*L_PAD padded node slots.
 - bf16 data path: p table, gathers, S matrices, hT all bf16 (PSUM math fp32).
   Table rows padded to 384 cols so gather elem bytes (768) is a 256 multiple.
 - Per layer: p = h @ W computed for own nodes (kept in SBUF + written to
   DRAM), AllGather -> replicated p table, per-edge gather of p[src] rows via
   dma_gather (edges sharded by dst core, grouped by 128-dst block),
   aggregation as PE matmuls with host-precomputed weighted one-hot selection
   matrices S (norm folded in, bias via a rank-1 mask-matmul), self-loop term
   from the SBUF-resident p_own, relu on evacuation, transpose to
   feature-major for the next layer's lhsT.
 - Pooling: per-graph column-slices reduce_max (pad cols are exactly 0, relu
   output >= 0, so padding never changes the max). Pooled vectors AllGathered,
   MLP head computed redundantly on every core.
"""
import os
import sys
import numpy as np

for _p in ('/opt/trn_rl_repo', '/root/.axon_site/_ro/trn_rl_repo'):
    if os.path.isdir(_p) and _p not in sys.path:
        sys.path.insert(0, _p)

N_CORES = 8
N_NODES = 50000
D = 320
DP = 384  # padded table row (bf16 -> 768B, multiple of 256)
DP8 = 512  # padded table row (fp8 -> 512B, multiple of 256)
N_GRAPHS = 64
GPC = N_GRAPHS // N_CORES  # graphs per core
# per-layer gather-table dtype: 'f8' (e4m3, 512B rows) or 'bf16' (768B rows)
TABLE_DTS = ('f8', 'f8', 'f8')


def _preprocess(x, edge_index, batch):
    """Build per-core gather indices, selection matrices and layouts."""
    batch = np.asarray(batch).astype(np.int64)
    src = np.asarray(edge_index[0]).astype(np.int64)
    dst = np.asarray(edge_index[1]).astype(np.int64)
    counts = np.bincount(batch, minlength=N_GRAPHS)
    L_PAD = max(896, int(-(-counts.max() // 128)) * 128)
    M_PAD = GPC * L_PAD
    TOT = N_CORES * M_PAD
    HALF = TOT // 2
    assert HALF <= 32767, (L_PAD, HALF)
    NB = M_PAD // 128  # dst blocks per core

    gstart = np.zeros(N_GRAPHS, np.int64)
    gstart[1:] = np.cumsum(counts)[:-1]
    n_ar = np.arange(N_NODES, dtype=np.int64)
    # within-graph relabeling: deal nodes (sorted by in-degree desc) round-robin
    # into the graph's L_PAD/128 blocks so per-block in-degree is balanced,
    # which minimizes the uniform per-(block,half) tile count K_FIX.
    NBLK = L_PAD // 128
    indeg = np.bincount(dst, minlength=N_NODES) + 1
    order_bal = np.lexsort((-indeg, batch))
    r = n_ar - gstart[batch[order_bal]]
    posg = (r % NBLK) * 128 + r // NBLK
    pos_in_graph = np.empty(N_NODES, np.int64)
    pos_in_graph[order_bal] = posg
    pos = (batch // GPC) * M_PAD + (batch % GPC) * L_PAD + pos_in_graph

    deg = np.bincount(dst, minlength=N_NODES).astype(np.float64) + 1.0
    dinv = 1.0 / np.sqrt(deg)

    # self-loops are applied locally (h += dinv^2 * p_own), not gathered
    ms = src
    mt = dst
    w = (dinv[src] * dinv[dst]).astype(np.float32)

    ms_pos = pos[ms]
    mt_pos = pos[mt]
    core = mt_pos // M_PAD
    lb = (mt_pos % M_PAD) // 128
    dl = mt_pos % 128
    half = ms_pos // HALF
    idxl = (ms_pos % HALF).astype(np.int64)

    key = (core * NB + lb) * 2 + half
    order = np.argsort(key, kind='stable')
    key_s = key[order]
    idxl_s = idxl[order]
    dl_s = dl[order]
    w_s = w[order]

    nkeys = N_CORES * NB * 2
    kcounts = np.bincount(key_s, minlength=nkeys).reshape(N_CORES, NB, 2)
    k_req = -(-kcounts // 128)  # ceil
    K_FIX = k_req.max(axis=0)  # [NB, 2] uniform across cores
    toff = np.zeros((NB, 2), np.int64)
    flat_k = K_FIX.reshape(-1)
    toff.reshape(-1)[1:] = np.cumsum(flat_k)[:-1]
    T_TOTAL = int(flat_k.sum())

    # rank of each message within its (core, lb, half) group
    kstart = np.zeros(nkeys, np.int64)
    kstart[1:] = np.cumsum(np.bincount(key_s, minlength=nkeys))[:-1]
    rank = np.arange(len(key_s)) - kstart[key_s]

    core_s = key_s // (NB * 2)
    lbh = key_s % (NB * 2)
    lb_s = lbh // 2
    half_s = lbh % 2
    t_glob = toff[lb_s, half_s] + rank // 128  # global tile id
    p_slot = rank % 128

    S_all = np.zeros((N_CORES, 128, T_TOTAL * 128), np.float32)
    S_all[core_s, p_slot, t_glob * 128 + dl_s] = w_s

    # dma_gather flat order: flat[i] = idxs[i % 16, i // 16]; i is the
    # message index within the call starting at tile toff[lb, half].
    idx_all = np.zeros((N_CORES, 16, T_TOTAL * 8), np.int16)
    colbase = toff[lb_s, half_s] * 8
    idx_all[core_s, rank % 16, colbase + rank // 16] = idxl_s.astype(np.int16)
    idx_rep = np.tile(idx_all, (1, 8, 1))  # [N_CORES, 128, T*8]

    # real-node mask per core [1, M_PAD]
    real = np.zeros(TOT, np.float32)
    real[pos] = 1.0
    mask = real.reshape(N_CORES, 1, M_PAD)

    # dinv^2 per padded slot, laid out [core, 128, NB] (partition p, block b)
    d2_flat = np.zeros(TOT, np.float32)
    d2_flat[pos] = (dinv * dinv).astype(np.float32)
    dinv2 = d2_flat.reshape(N_CORES, NB, 128).transpose(0, 2, 1).copy()

    # xT padded per core [D, M_PAD]
    x = np.asarray(x, dtype=np.float32)
    xT_pad = np.zeros((D, TOT), np.float32)
    xT_pad[:, pos] = x.T
    xT_own = np.stack([xT_pad[:, c * M_PAD:(c + 1) * M_PAD] for c in range(N_CORES)])

    meta = dict(L_PAD=L_PAD, M_PAD=M_PAD, TOT=TOT, HALF=HALF, NB=NB,
                K_FIX=K_FIX, toff=toff, T_TOTAL=T_TOTAL)
    return meta, S_all, idx_rep, mask, xT_own, dinv2


def _build_bass(meta, weights, repeat=1, ablate=()):
    from concourse import mybir, bacc
    import concourse.tile as tile
    from concourse.masks import make_identity

    L_PAD = meta['L_PAD']; M_PAD = meta['M_PAD']; TOT = meta['TOT']
    HALF = meta['HALF']; NB = meta['NB']
    K_FIX = meta['K_FIX']; toff = meta['toff']; T_TOTAL = meta['T_TOTAL']
    f32 = mybir.dt.float32
    bf16 = mybir.dt.bfloat16
    f8 = mybir.dt.float8e4
    i16 = mybir.dt.int16
    TDT = [f8 if t == 'f8' else bf16 for t in TABLE_DTS]
    TDP = [DP8 if t == 'f8' else DP for t in TABLE_DTS]
    FCH = [(0, 128), (128, 128), (256, 64)]  # feature chunks of 320

    nc = bacc.Bacc("TRN2", target_bir_lowering=False, debug=False,
                   num_devices=N_CORES, num_swdge_queues=4)

    # ---- IO ----
    S_t = nc.dram_tensor("S_all", [128, T_TOTAL * 128], f8, kind="ExternalInput")
    idx_t = nc.dram_tensor("idx_all", [128, T_TOTAL * 8], i16, kind="ExternalInput")
    mask_t = nc.dram_tensor("mask", [1, M_PAD], bf16, kind="ExternalInput")
    dinv2_t = nc.dram_tensor("dinv2", [128, NB], f32, kind="ExternalInput")
    xT_t = nc.dram_tensor("xT", [D, M_PAD], bf16, kind="ExternalInput")
    W_t = [nc.dram_tensor(f"W{k}", [D, D], bf16, kind="ExternalInput") for k in (1, 2, 3)]
    brow_t = [nc.dram_tensor(f"brow{k}", [1, D], bf16, kind="ExternalInput") for k in (1, 2, 3)]
    Wf1_t = nc.dram_tensor("Wf1", [320, 256], f32, kind="ExternalInput")
    bf1_t = nc.dram_tensor("bf1c", [128, 2], f32, kind="ExternalInput")
    Wf2_t = nc.dram_tensor("Wf2", [256, 16], f32, kind="ExternalInput")
    bf2_t = nc.dram_tensor("bf2c", [16, 1], f32, kind="ExternalInput")
    Wf3_t = nc.dram_tensor("Wf3", [16, 1], f32, kind="ExternalInput")
    out_t = nc.dram_tensor("out", [1, N_GRAPHS], f32, kind="ExternalOutput")
    bf3_val = float(np.asarray(weights['bf3']).reshape(-1)[0])

    p_own = [nc.dram_tensor(f"p_own{k}", [M_PAD, TDP[k]], TDT[k], kind="Internal")
             for k in range(3)]
    p_full = [nc.dram_tensor(f"p_full{k}", [TOT, TDP[k]], TDT[k], kind="Internal",
                             addr_space="Shared") for k in range(3)]
    pooled_own = nc.dram_tensor("pooled_own", [D, GPC], f32, kind="Internal")
    pooled_all = nc.dram_tensor("pooled_all", [N_CORES, D, GPC], f32,
                                kind="Internal", addr_space="Shared")

    RG = [list(range(N_CORES))]
    KMAXB = int((K_FIX[:, 0] + K_FIX[:, 1]).max())

    with tile.TileContext(nc) as tc:
        with tc.tile_pool(name="persist", bufs=1) as pp, \
             tc.tile_pool(name="gpool", bufs=2) as gp, \
             tc.tile_pool(name="spool", bufs=2) as sp, \
             tc.tile_pool(name="hpool", bufs=2) as hp, \
             tc.tile_pool(name="psum_a", bufs=2, space="PSUM") as pa, \
             tc.tile_pool(name="psum_t", bufs=2, space="PSUM") as pt, \
             tc.tile_pool(name="psum_f", bufs=2, space="PSUM") as pf:

            ident_f = pp.tile([128, 128], f32, tag="ident_f")
            make_identity(nc, ident_f[:])
            ident = pp.tile([128, 128], bf16, tag="ident")
            nc.vector.tensor_copy(ident[:], ident_f[:])
            idx_sb = pp.tile([128, T_TOTAL * 8], i16, tag="idx")
            nc.sync.dma_start(idx_sb[:], idx_t[:, :])
            d2_sb = pp.tile([128, NB], f32, tag="d2")
            nc.sync.dma_start(d2_sb[:], dinv2_t[:, :])
            brow_sb = pp.tile([1, 3 * D], bf16, tag="brow")
            for k in range(3):
                nc.sync.dma_start(brow_sb[:, k * D:(k + 1) * D], brow_t[k][:, :])
            mask_sb = pp.tile([1, M_PAD], bf16, tag="mask")
            nc.sync.dma_start(mask_sb[:], mask_t[:, :])

            # persistent transposed features hT (3 chunks)
            hT = [pp.tile([cl, M_PAD], bf16, tag=f"hT{ci}", name=f"hT{ci}")
                  for ci, (cs, cl) in enumerate(FCH)]
            for ci, (cs, cl) in enumerate(FCH):
                nc.sync.dma_start(hT[ci][:], xT_t[cs:cs + cl, :])

            # persistent own-p (this layer's p for own nodes), node-major
            pown_sb = pp.tile([128, NB * D], bf16, tag="pown")

            # W chunks for the current layer (rhs [128,320] x3), reloaded per layer
            def feature_matmul_phase(layer):
                """p_own[layer] = h @ W[layer] for own nodes (h given by hT)."""
                wl = []
                for ci, (cs, cl) in enumerate(FCH):
                    wt = hp.tile([cl, D], bf16, tag=f"wch{ci}", name=f"wch{ci}", bufs=1)
                    nc.sync.dma_start(wt[:], W_t[layer][cs:cs + cl, :])
                    wl.append(wt)
                for nb in range(NB):
                    ps = pf.tile([128, D], f32, tag="pfeat")
                    for ci, (cs, cl) in enumerate(FCH):
                        nc.tensor.matmul(
                            ps[:],
                            lhsT=hT[ci][:, nb * 128:(nb + 1) * 128],
                            rhs=wl[ci][:],
                            start=(ci == 0), stop=(ci == 2))
                    nc.any.tensor_copy(pown_sb[:, nb * D:(nb + 1) * D], ps[:])
                    if TDT[layer] == bf16:
                        nc.sync.dma_start(
                            p_own[layer][nb * 128:(nb + 1) * 128, 0:D],
                            pown_sb[:, nb * D:(nb + 1) * D])
                    else:
                        pq = hp.tile([128, D], f8, tag="pq")
                        nc.vector.tensor_copy(pq[:], ps[:])
                        nc.sync.dma_start(
                            p_own[layer][nb * 128:(nb + 1) * 128, 0:D], pq[:])
                if 'ag' not in ablate:
                    nc.gpsimd.collective_compute(
                        "AllGather", mybir.AluOpType.bypass, replica_groups=RG,
                        ins=[p_own[layer][:, :]], outs=[p_full[layer][:, :]])

            qn = [0]

            def agg_phase(layer):
                """hT = relu(Ahat @ p_full[layer] + b) transposed, per dst block."""
                table = p_full[layer]
                ldt, ldp = TDT[layer], TDP[layer]
                for b in range(NB):
                    k0, k1 = int(K_FIX[b, 0]), int(K_FIX[b, 1])
                    ktot = k0 + k1
                    t0 = int(toff[b, 0])
                    g = gp.tile([128, KMAXB, ldp], ldt, tag="g")
                    if 'gather' in ablate:
                        nc.vector.memset(g[:, 0, 0:8], 0.125)
                    for hh, (kh, th) in enumerate(((k0, int(toff[b, 0])),
                                                   (k1, int(toff[b, 1])))):
                        if kh == 0 or 'gather' in ablate:
                            continue
                        koff = 0 if hh == 0 else k0
                        nc.gpsimd.dma_gather(
                            out_ap=g[:, koff:koff + kh, :],
                            in_ap=table[hh * HALF:(hh + 1) * HALF, :],
                            idxs_ap=idx_sb[:, th * 8:(th + kh) * 8],
                            num_idxs=kh * 128,
                            num_idxs_reg=kh * 128,
                            elem_size=ldp,
                            single_packet=False,
                            queue_num=qn[0] % 4)
                        qn[0] += 1
                    s_sb = sp.tile([128, KMAXB * 128], f8, tag="s")
                    if 'sload' not in ablate:
                        nc.sync.dma_start(s_sb[:, :ktot * 128],
                                          S_t[:, t0 * 128:(t0 + ktot) * 128])
                    else:
                        nc.vector.memset(s_sb[:, 0:8], 0.0)
                    ps = pa.tile([128, D], f32, tag="pagg")
                    nc.tensor.matmul(
                        ps[:],
                        lhsT=mask_sb[:, b * 128:(b + 1) * 128],
                        rhs=brow_sb[:, layer * D:(layer + 1) * D],
                        start=True, stop=False)
                    for t in range(ktot):
                        if 'smm' in ablate:
                            break
                        nc.tensor.matmul(
                            ps[:],
                            lhsT=s_sb[:, t * 128:(t + 1) * 128],
                            rhs=g[:, t, 0:D],
                            start=False, stop=(t == ktot - 1))
                    if 'smm' in ablate:
                        nc.tensor.matmul(
                            ps[:], lhsT=s_sb[:, 0:128], rhs=g[:, 0, 0:D],
                            start=False, stop=True)
                    slt = hp.tile([128, D], f32, tag="slt",
                                  name=f"slt_{layer}_{b}")
                    nc.vector.tensor_scalar_mul(slt[:],
                                                pown_sb[:, b * D:(b + 1) * D],
                                                d2_sb[:, b:b + 1])
                    htmp = hp.tile([128, D], bf16, tag="htmp")
                    nc.vector.tensor_tensor(out=htmp[:], in0=ps[:], in1=slt[:],
                                            op=mybir.AluOpType.add)
                    nc.vector.tensor_scalar_max(htmp[:], htmp[:], 0.0)
                    for ci, (cs, cl) in enumerate(FCH):
                        tp = pt.tile([128, 128], bf16, tag="tr")
                        nc.tensor.transpose(tp[:cl, :], htmp[:, cs:cs + cl],
                                            identity=ident[:])
                        nc.any.tensor_copy(hT[ci][:, b * 128:(b + 1) * 128],
                                           tp[:cl, :])

            # ---- network ----
            import contextlib
            loop_ctx = tc.For_i(0, repeat, 1) if repeat > 1 else contextlib.nullcontext()
            with loop_ctx:
                if 'layers' not in ablate:
                    feature_matmul_phase(0)  # p1 = x @ W1
                    agg_phase(0)             # h1
                    feature_matmul_phase(1)  # p2 = h1 @ W2
                    agg_phase(1)             # h2
                    feature_matmul_phase(2)  # p3 = h2 @ W3
                    agg_phase(2)             # h3 (lives in hT)

            # ---- global max pool ----
            for ci, (cs, cl) in enumerate(FCH):
                gt = hp.tile([cl, GPC], f32, tag=f"gt{ci}", name=f"gt{ci}", bufs=1)
                for j in range(GPC):
                    nc.vector.reduce_max(
                        gt[:, j:j + 1], hT[ci][:, j * L_PAD:(j + 1) * L_PAD],
                        axis=mybir.AxisListType.X)
                nc.sync.dma_start(pooled_own[cs:cs + cl, :], gt[:])
            nc.gpsimd.collective_compute(
                "AllGather", mybir.AluOpType.bypass, replica_groups=RG,
                ins=[pooled_own[:, :]], outs=[pooled_all[:, :, :]])

            # gT_full chunks [cl, 64]
            gT = []
            for ci, (cs, cl) in enumerate(FCH):
                gtile = hp.tile([cl, N_GRAPHS], f32, tag=f"gTf{ci}", name=f"gTf{ci}", bufs=1)
                for cc in range(N_CORES):
                    nc.sync.dma_start(gtile[:, cc * GPC:(cc + 1) * GPC],
                                      pooled_all[cc, cs:cs + cl, :])
                gT.append(gtile)

            # ---- MLP head (transposed): z1T[256,64] ----
            wf1 = []
            for mi in range(2):
                for ci, (cs, cl) in enumerate(FCH):
                    t = hp.tile([cl, 128], f32, tag=f"wf1_{mi}_{ci}", name=f"wf1_{mi}_{ci}", bufs=1)
                    nc.sync.dma_start(t[:], Wf1_t[cs:cs + cl, mi * 128:(mi + 1) * 128])
                    wf1.append(t)
            bf1sb = hp.tile([128, 2], f32, tag="bf1", bufs=1)
            nc.sync.dma_start(bf1sb[:], bf1_t[:, :])
            h1T = []
            for mi in range(2):
                ps = pf.tile([128, N_GRAPHS], f32, tag="pfeat")
                for ci in range(3):
                    nc.tensor.matmul(ps[:], lhsT=wf1[mi * 3 + ci][:],
                                     rhs=gT[ci][:],
                                     start=(ci == 0), stop=(ci == 2))
                h = hp.tile([128, N_GRAPHS], f32, tag=f"h1T{mi}", name=f"h1T{mi}", bufs=1)
                nc.vector.tensor_scalar(h[:], ps[:],
                                        bf1sb[:, mi:mi + 1], 0.0,
                                        op0=mybir.AluOpType.add,
                                        op1=mybir.AluOpType.max)
                h1T.append(h)
            # z2T [16, 64]
            wf2 = []
            for mi in range(2):
                t = hp.tile([128, 16], f32, tag=f"wf2_{mi}", name=f"wf2_{mi}", bufs=1)
                nc.sync.dma_start(t[:], Wf2_t[mi * 128:(mi + 1) * 128, :])
                wf2.append(t)
            bf2sb = hp.tile([16, 1], f32, tag="bf2", bufs=1)
            nc.sync.dma_start(bf2sb[:], bf2_t[:, :])
            ps2 = pf.tile([16, N_GRAPHS], f32, tag="pfeat")
            for mi in range(2):
                nc.tensor.matmul(ps2[:], lhsT=wf2[mi][:],
                                 rhs=h1T[mi][:],
                                 start=(mi == 0), stop=(mi == 1))
            h2T = hp.tile([16, N_GRAPHS], f32, tag="h2T", bufs=1)
            nc.vector.tensor_scalar(h2T[:], ps2[:], bf2sb[:, 0:1], 0.0,
                                    op0=mybir.AluOpType.add,
                                    op1=mybir.AluOpType.max)
            # z3 [1, 64]
            wf3 = hp.tile([16, 1], f32, tag="wf3", bufs=1)
            nc.sync.dma_start(wf3[:], Wf3_t[:, :])
            ps3 = pf.tile([1, N_GRAPHS], f32, tag="pfeat")
            nc.tensor.matmul(ps3[:], lhsT=wf3[:],
                             rhs=h2T[:], start=True, stop=True)
            osb = hp.tile([1, N_GRAPHS], f32, tag="osb", bufs=1)
            nc.vector.tensor_scalar(osb[:], ps3[:], bf3_val, None,
                                    op0=mybir.AluOpType.add)
            nc.sync.dma_start(out_t[:, :], osb[:])

    nc.compile()
    return nc


def _make_runner(nc, in_maps):
    """Build a reusable jitted SPMD executor for `nc` (axon/PJRT path).

    Returns (run_fn, out_names, out_avals): run_fn() executes once and
    returns the list of per-core result dicts.
    """
    import jax
    import numpy as np
    from jax.experimental.shard_map import shard_map
    from jax.sharding import Mesh, NamedSharding, PartitionSpec
    from concourse import bass2jax, mybir

    bass2jax.install_neuronx_cc_hook()
    n_cores = len(in_maps)
    partition_name = nc.partition_id_tensor.name if nc.partition_id_tensor else None
    in_names, out_names, out_avals, zero_outs = [], [], [], []
    for alloc in nc.m.functions[0].allocations:
        if not isinstance(alloc, mybir.MemoryLocationSet):
            continue
        name = alloc.memorylocations[0].name
        if alloc.kind == "ExternalInput":
            if name != partition_name:
                in_names.append(name)
        elif alloc.kind == "ExternalOutput":
            shape = tuple(alloc.tensor_shape)
            dtype = mybir.dt.np(alloc.dtype)
            out_names.append(name)
            out_avals.append(jax.core.ShapedArray(shape, dtype))
            zero_outs.append(np.zeros(shape, dtype))
    n_params = len(in_names)
    n_outs = len(out_avals)
    all_in_names = list(in_names) + list(out_names)
    if partition_name is not None:
        all_in_names.append(partition_name)
    donate = tuple(range(n_params, n_params + n_outs))

    def _body(*args):
        operands = list(args)
        if partition_name is not None:
            operands.append(bass2jax.partition_id_tensor())
        outs = bass2jax._bass_exec_p.bind(
            *operands,
            out_avals=tuple(out_avals),
            in_names=tuple(all_in_names),
            out_names=tuple(out_names),
            lowering_input_output_aliases=(),
            sim_require_finite=True,
            sim_require_nnan=True,
            nc=nc,
        )
        return tuple(outs)

    devices = jax.devices()[:n_cores]
    mesh = Mesh(np.asarray(devices), ("core",))
    in_specs = (PartitionSpec("core"),) * (n_params + n_outs)
    out_specs = (PartitionSpec("core"),) * len(out_names)
    sharded = jax.jit(
        shard_map(_body, mesh=mesh, in_specs=in_specs, out_specs=out_specs,
                  check_rep=False),
        donate_argnums=donate, keep_unused=True)
    sh = NamedSharding(mesh, PartitionSpec("core"))
    concat_in = [
        jax.device_put(
            np.concatenate([np.asarray(in_maps[c][nm]) for c in range(n_cores)],
                           axis=0), sh)
        for nm in in_names
    ]

    def run_fn():
        zeros = [np.zeros((n_cores * z.shape[0], *z.shape[1:]), z.dtype)
                 for z in zero_outs]
        out_arrs = sharded(*concat_in, *zeros)
        out_arrs = [np.asarray(o) for o in out_arrs]
        return [
            {nm: out_arrs[i].reshape(n_cores, *out_avals[i].shape)[c]
             for i, nm in enumerate(out_names)}
            for c in range(n_cores)
        ]

    return run_fn, out_names, out_avals


def prepare(inputs, repeat=1, ablate=()):
    """Preprocess + build + compile; returns a reusable run_fn."""
    meta, S_all, idx_rep, mask, xT_own, dinv2 = _preprocess(
        inputs['x'], inputs['edge_index'], inputs['batch'])
    nc = _build_bass(meta, inputs, repeat=repeat, ablate=ablate)
    in_maps = _make_in_maps(inputs, S_all, idx_rep, mask, xT_own, dinv2)
    run_fn, _, _ = _make_runner(nc, in_maps)
    return run_fn


def _make_in_maps(inputs, S_all, idx_rep, mask, xT_own, dinv2):
    import ml_dtypes
    bf = ml_dtypes.bfloat16
    f8 = ml_dtypes.float8_e4m3fn
    in_maps = []
    for c in range(N_CORES):
        m = {
            "S_all": S_all[c].astype(f8),
            "idx_all": idx_rep[c],
            "mask": mask[c].astype(bf),
            "dinv2": np.ascontiguousarray(dinv2[c]),
            "xT": np.ascontiguousarray(xT_own[c]).astype(bf),
            "Wf1": np.asarray(inputs['Wf1'], np.float32),
            "bf1c": np.ascontiguousarray(
                np.asarray(inputs['bf1'], np.float32).reshape(2, 128).T),
            "Wf2": np.asarray(inputs['Wf2'], np.float32),
            "bf2c": np.asarray(inputs['bf2'], np.float32).reshape(16, 1),
            "Wf3": np.asarray(inputs['Wf3'], np.float32),
        }
        for k in (1, 2, 3):
            m[f"W{k}"] = np.asarray(inputs[f'W{k}'], np.float32).astype(bf)
            m[f"brow{k}"] = np.asarray(inputs[f'b{k}'], np.float32).reshape(1, D).astype(bf)
        in_maps.append(m)
    return in_maps


def kernel(**inputs):
    meta, S_all, idx_rep, mask, xT_own, dinv2 = _preprocess(
        inputs['x'], inputs['edge_index'], inputs['batch'])
    nc = _build_bass(meta, inputs)
    in_maps = _make_in_maps(inputs, S_all, idx_rep, mask, xT_own, dinv2)
    from concourse.bass_utils import run_bass_kernel_spmd
    res = run_bass_kernel_spmd(nc, in_maps, core_ids=list(range(N_CORES)),
                               trace=False)
    out = np.asarray(res.results[0]["out"]).reshape(1, N_GRAPHS)
    return out.T.copy()
